# revision 10
# baseline (speedup 1.0000x reference)
"""GroupedQueryAttention Trainium2 kernel (8 NeuronCores).

Sharding: core c -> q-heads [4c,4c+4), kv-head c, BOTH batches (tensor
parallel 8-way on heads). Output tokens: batch c//4, slice 512*(c%4).

Per core: qkv projection (bias via ones-row 17th contraction chunk) + RoPE;
flash-style causal attention in transposed-score orientation probsT[k,q],
head-PAIRED on the PE: pair p in {0,1} holds heads (2p, 2p+1) in partition
halves, batch b processed as separate super-unit (p,b). The kv head is
shared by both halves, so kT2 is duplicated across partition halves via an
SBUF->SBUF DMA. Denominators via a ones-column appended to V. Exp split:
half 0 -> ScalarE activation, half 1 -> VectorE int16 Schraudolph
(bf16_bits = int16(score*A + B)). Normalization per 512-token PV chunk:
reciprocal_approx_fast of the PSUM den row + gpsimd partition_broadcast +
multiply, sent straight to the AllToAll input (head-split -> token-split;
every A2A byte useful, no blending). A2A#1 (heads 0,1) fires mid-attention;
A2A#2 (heads 2,3) right after the last chunk; o_proj GEMM for the first
head-half hides A2A#2. wo is streamed from DRAM (no SBUF residency).

Self-contained: hardcodes all shapes; only imports the concourse toolchain.
"""

import sys

for _p in ("/opt/trn_rl_repo", "/root/.axon_site/_ro/trn_rl_repo"):
    if _p not in sys.path:
        sys.path.insert(0, _p)

import math

import numpy as np
import ml_dtypes

import concourse.bass as bass
import concourse.mybir as mybir
import concourse.tile as tile
from concourse import bacc
from concourse.bass_utils import run_bass_kernel_spmd
from concourse.masks import make_identity

B, S, HID = 2, 2048, 2048
NH, NKV, HD = 32, 8, 64
GROUPS = NH // NKV
ROPE_BASE = 10000.0
NCORES = 8

BF = mybir.dt.bfloat16
F32 = mybir.dt.float32
I16 = mybir.dt.int16

NB = S // 128  # 16 k strip blocks
LQ = [S - 128 * j for j in range(NB)]
OFFX = [0]
for _j in range(NB):
    OFFX.append(OFFX[-1] + LQ[_j])
TOT = OFFX[-1]  # 17408

LOG2E = 1.4426950408889634
SCALE = 1.0 / math.sqrt(HD)
SCH_A = SCALE * LOG2E * 128.0
SCH_B = 127.0 * 128.0 - 5.6

_CACHED = {}


def _build_nc():
    nc = bacc.Bacc("TRN2", target_bir_lowering=False, debug=False,
                   num_devices=NCORES)

    hsT = nc.declare_dram_parameter("hsT", [2, 17, 128, S], BF, isOutput=False)
    wq = nc.declare_dram_parameter("wq", [17, 128, 256], BF, isOutput=False)
    wkv0 = nc.declare_dram_parameter("wkv0", [17, 128, 128], BF,
                                     isOutput=False)
    wkv1 = nc.declare_dram_parameter("wkv1", [17, 128, 128], BF,
                                     isOutput=False)
    wo = nc.declare_dram_parameter("wo", [16, 128, HID], BF, isOutput=False)
    cos2 = nc.declare_dram_parameter("cos2", [128, S], BF, isOutput=False)
    sin2 = nc.declare_dram_parameter("sin2", [128, S], BF, isOutput=False)
    mb = nc.declare_dram_parameter("maskbin", [128, 128], BF, isOutput=False)
    out_part = nc.declare_dram_parameter("out_part", [512, HID], BF,
                                         isOutput=True)

    MULT = mybir.AluOpType.mult
    ADD = mybir.AluOpType.add
    EXP = mybir.ActivationFunctionType.Exp

    with tile.TileContext(nc) as tc:
        with tc.tile_pool(name="pers", bufs=1) as pers, \
             tc.tile_pool(name="dram", bufs=1, space="DRAM") as dram:
            # qT2[dim-of-head-pair, pair, batch, pos]; halves = heads 2p/2p+1
            qT2 = pers.tile([128, 2, 2, S], BF)
            # kT2[dup kv dims (both halves identical), batch, pos]
            kT2 = pers.tile([128, 2, S], BF)
            # v_aug[kpos, batch, block, dim+ones]
            v_aug = pers.tile([128, 2, NB, 65], BF)
            idn = pers.tile([128, 128], BF)
            make_identity(nc, idn)
            maskb = pers.tile([128, 128], BF)
            nc.gpsimd.dma_start(out=maskb[:], in_=mb[:])
            nc.vector.memset(v_aug[:, :, :, 64:65], 1.0)
            # gathered attention rows for o_proj (one tile per A2A)
            attg0 = pers.tile([128, 8, 512], BF)
            attg1 = pers.tile([128, 8, 512], BF)

            a2aA_in = dram.tile([NCORES, 128, 512], BF)
            a2aA_out = dram.tile([NCORES, 128, 512], BF)
            a2aB_in = dram.tile([NCORES, 128, 512], BF)
            a2aB_out = dram.tile([NCORES, 128, 512], BF)

            # ---------------- qkv projection + RoPE (bf16) ----------------
            with tc.tile_pool(name="proj", bufs=1) as pj, \
                 tc.tile_pool(name="projp", bufs=1, space="PSUM") as pjp:
                hsT_sb = pj.tile([128, 2, 17, S], BF)
                cos_sb = pj.tile([128, S], BF)
                sin_sb = pj.tile([128, S], BF)
                vT2 = pj.tile([128, S], BF)  # rows: [v(b1) | v(b0)]

                # startup loads spread over the 3 DMA trigger queues; order
                # matches consumption (kv(b0) first)
                for ki in range(17):
                    nc.sync.dma_start(out=hsT_sb[:, 0, ki, :],
                                      in_=hsT[0, ki])
                    nc.scalar.dma_start(out=hsT_sb[:, 1, ki, :],
                                        in_=hsT[1, ki])
                nc.gpsimd.dma_start(out=cos_sb[:], in_=cos2[:])
                nc.gpsimd.dma_start(out=sin_sb[:], in_=sin2[:])

                def rope(pq, lo, hi, dst_slices, sl):
                    """RoPE rows [lo:hi) of psum pq; write to dst slices."""
                    n = hi - lo
                    tmp = pj.tile([128, 512], BF, tag="ropetmp", bufs=3)
                    pairs = [(0, 32), (32, 0), (64, 96), (96, 64)]
                    for (a, bb) in pairs:
                        if a >= n:
                            continue
                        nc.vector.tensor_tensor(
                            out=tmp[lo + a:lo + a + 32, :],
                            in0=pq[lo + bb:lo + bb + 32, :],
                            in1=sin_sb[lo + a:lo + a + 32, sl], op=MULT)
                    tmp2 = pj.tile([128, 512], BF, tag="ropetmp2", bufs=3)
                    nc.vector.tensor_tensor(out=tmp2[lo:hi, :],
                                            in0=pq[lo:hi, :],
                                            in1=cos_sb[lo:hi, sl], op=MULT)
                    for dst in dst_slices:
                        nc.vector.tensor_tensor(out=dst, in0=tmp2[lo:hi, :],
                                                in1=tmp[lo:hi, :], op=ADD)

                # kv units: b0 uses [k|v] weights, b1 uses [v|k]
                for b in range(2):
                    wkv_src = wkv0 if b == 0 else wkv1
                    accs = [pjp.tile([128, 512], F32, tag="acc", bufs=6,
                                     name=f"acckv_{b}_{Q}") for Q in range(4)]
                    for ki in range(17):
                        wt = pj.tile([128, 128], BF, tag="wkv", bufs=4,
                                     name=f"wkv_{b}_{ki}")
                        nc.gpsimd.dma_start(out=wt[:], in_=wkv_src[ki])
                        for Q in range(4):
                            nc.tensor.matmul(
                                accs[Q][:], lhsT=wt[:],
                                rhs=hsT_sb[:, b, ki, 512 * Q:512 * Q + 512],
                                start=(ki == 0), stop=(ki == 16))
                    for Q in range(4):
                        sl = slice(512 * Q, 512 * Q + 512)
                        pq = accs[Q]
                        if b == 0:
                            # rows 0:64 = k(b0) -> kT2[0:64, 0]
                            rope(pq, 0, 64, [kT2[0:64, 0, sl]], sl)
                            nc.scalar.copy(vT2[64:128, sl], pq[64:128, :])
                        else:
                            # rows 64:128 = k(b1) -> kT2[64:128, 1]
                            rope(pq, 64, 128, [kT2[64:128, 1, sl]], sl)
                            nc.scalar.copy(vT2[0:64, sl], pq[0:64, :])
                    # duplicate kv head across partition halves
                    # (partition-shifted DVE copies, 512 cols each)
                    for Q in range(4):
                        sl = slice(512 * Q, 512 * Q + 512)
                        if b == 0:
                            nc.vector.tensor_copy(kT2[64:128, 0, sl],
                                                  kT2[0:64, 0, sl])
                        else:
                            nc.vector.tensor_copy(kT2[0:64, 1, sl],
                                                  kT2[64:128, 1, sl])

                # q units: pair p holds heads (2p, 2p+1) in halves
                for p in range(2):
                    for b in range(2):
                        accs = [pjp.tile([128, 512], F32, tag="acc", bufs=6,
                                         name=f"accq_{p}_{b}_{Q}")
                                for Q in range(4)]
                        for ki in range(17):
                            wt = pj.tile([128, 128], BF, tag="wq", bufs=4,
                                         name=f"wq_{p}_{b}_{ki}")
                            nc.scalar.dma_start(
                                out=wt[:],
                                in_=wq[ki, :, 128 * p:128 * p + 128])
                            for Q in range(4):
                                nc.tensor.matmul(
                                    accs[Q][:], lhsT=wt[:],
                                    rhs=hsT_sb[:, b, ki,
                                               512 * Q:512 * Q + 512],
                                    start=(ki == 0), stop=(ki == 16))
                        for Q in range(4):
                            sl = slice(512 * Q, 512 * Q + 512)
                            rope(accs[Q], 0, 128, [qT2[:, p, b, sl]], sl)

                # v: vT2 rows [v(b1)|v(b0)] -> v_aug[kpos, batch, block, dim]
                for kb in range(NB):
                    pvt = pjp.tile([128, 128], BF, tag="vt", bufs=2,
                                   name=f"pvt_{kb}")
                    nc.tensor.transpose(pvt[:], vT2[:, 128 * kb:128 * kb + 128],
                                        idn[:])
                    nc.vector.tensor_copy(v_aug[:, 1, kb, 0:64], pvt[:, 0:64])
                    nc.vector.tensor_copy(v_aug[:, 0, kb, 0:64],
                                          pvt[:, 64:128])

            # ---------------- attention ----------------
            with tc.tile_pool(name="att", bufs=1) as at, \
                 tc.tile_pool(name="attp", bufs=1, space="PSUM") as atp:

                def qk_strip(p, b, j, probsL, probsH):
                    q0 = 128 * j
                    L = LQ[j]
                    for cb in range(0, L, 512):
                        w = min(512, L - cb)
                        for h, probs in ((0, probsL), (1, probsH)):
                            ps = atp.tile([128, 512], F32, tag="sc", bufs=5,
                                          name=f"sc_{p}_{b}_{j}_{cb}_{h}")
                            nc.tensor.matmul(
                                ps[:, 0:w],
                                lhsT=kT2[64 * h:64 * h + 64, b,
                                         q0:q0 + 128],
                                rhs=qT2[64 * h:64 * h + 64, p, b,
                                        q0 + cb:q0 + cb + w],
                                start=True, stop=True)
                            dst = probs[:, OFFX[j] + cb:OFFX[j] + cb + w]
                            if h == 0:
                                nc.scalar.activation(dst, ps[:, 0:w], EXP,
                                                     scale=SCALE)
                            else:
                                nc.vector.tensor_scalar(
                                    out=dst.bitcast(I16), in0=ps[:, 0:w],
                                    scalar1=SCH_A, scalar2=SCH_B,
                                    op0=MULT, op1=ADD)
                            if cb == 0:
                                # causal mask on the diagonal block
                                nc.vector.tensor_tensor(
                                    out=probs[:, OFFX[j]:OFFX[j] + 128],
                                    in0=probs[:, OFFX[j]:OFFX[j] + 128],
                                    in1=maskb[:], op=MULT)

                def pv_chunk(p, b, h, c, probs):
                    pvt = atp.tile([65, 512], F32, tag="pv", bufs=3,
                                   name=f"pv_{p}_{b}_{h}_{c}")
                    for j in range(4 * c + 4):
                        if j <= 4 * c:
                            col = OFFX[j] + 512 * c - 128 * j
                            nc.tensor.matmul(
                                pvt[:, 0:512], lhsT=v_aug[:, b, j, :],
                                rhs=probs[:, col:col + 512],
                                start=(j == 0), stop=(j == 4 * c + 3))
                        else:
                            d0 = 128 * (j - 4 * c)
                            nc.tensor.matmul(
                                pvt[:, d0:512], lhsT=v_aug[:, b, j, :],
                                rhs=probs[:, OFFX[j]:OFFX[j] + 512 - d0],
                                start=False, stop=(j == 4 * c + 3))
                    # normalize this 512-token slice and send to dest 4b+c
                    den = at.tile([1, 512], F32, tag="den", bufs=2,
                                  name=f"den_{p}_{b}_{h}_{c}")
                    nc.scalar.copy(den[:], pvt[64:65, :])
                    rec = at.tile([1, 512], F32, tag="rec", bufs=2,
                                  name=f"rec_{p}_{b}_{h}_{c}")
                    nc.vector.reciprocal_approx_fast(out=rec[:], in_=den[:])
                    rb = at.tile([64, 512], F32, tag="rb", bufs=2,
                                 name=f"rb_{p}_{b}_{h}_{c}")
                    nc.gpsimd.partition_broadcast(rb[:], rec[:])
                    an = at.tile([64, 512], BF, tag="an", bufs=2,
                                 name=f"an_{p}_{b}_{h}_{c}")
                    nc.vector.tensor_tensor(out=an[:], in0=pvt[0:64, :],
                                            in1=rb[:], op=MULT)
                    a2a_in = a2aA_in if p == 0 else a2aB_in
                    nc.sync.dma_start(
                        out=a2a_in[4 * b + c, 64 * h:64 * h + 64, :],
                        in_=an[:])

                for p in range(2):
                    for b in range(2):
                        probsL = at.tile([128, TOT], BF, tag="probsL", bufs=2,
                                         name=f"probsL_{p}_{b}")
                        probsH = at.tile([128, TOT], BF, tag="probsH", bufs=2,
                                         name=f"probsH_{p}_{b}")
                        for j in range(NB):
                            qk_strip(p, b, j, probsL, probsH)
                            if j % 4 == 3:
                                c = j // 4
                                pv_chunk(p, b, 0, c, probsL)
                                pv_chunk(p, b, 1, c, probsH)
                    # pair's 16 sends complete -> AllToAll
                    a2a_in = a2aA_in if p == 0 else a2aB_in
                    a2a_out = a2aA_out if p == 0 else a2aB_out
                    attg = attg0 if p == 0 else attg1
                    nc.gpsimd.collective_compute(
                        "AllToAll", mybir.AluOpType.bypass,
                        replica_groups=[list(range(NCORES))],
                        ins=[a2a_in.opt()], outs=[a2a_out.opt()])
                    nc.scalar.dma_start(
                        out=attg[:],
                        in_=a2a_out.rearrange("s p n -> p s n"))

            # ---------------- o_proj (my 512 tokens, all 2048 od) ---------
            # passes: (hf, od-half); wo streamed from DRAM; hf=0 GEMMs hide
            # A2A#2.
            with tc.tile_pool(name="op", bufs=1) as po, \
                 tc.tile_pool(name="opp", bufs=1, space="PSUM") as pop:
                part = {}
                for hf, attg in ((0, attg0), (1, attg1)):
                    for od in range(2):
                        ods = slice(1024 * od, 1024 * od + 1024)
                        psos = [pop.tile([128, 1024], F32, tag="po", bufs=4,
                                         name=f"pso_{hf}_{od}_{st}")
                                for st in range(4)]
                        for k8 in range(8):
                            wot = po.tile([128, 1024], BF, tag="wo", bufs=3,
                                          name=f"wo_{hf}_{od}_{k8}")
                            nc.sync.dma_start(out=wot[:],
                                              in_=wo[8 * hf + k8, :, ods])
                            for st in range(4):
                                lhsT = attg[:, k8, 128 * st:128 * st + 128]
                                for u in range(2):
                                    nc.tensor.matmul(
                                        psos[st][:, 512 * u:512 * u + 512],
                                        lhsT=lhsT,
                                        rhs=wot[:, 512 * u:512 * u + 512],
                                        start=(k8 == 0), stop=(k8 == 7))
                        for st in range(4):
                            if hf == 0:
                                pt = po.tile([128, 1024], BF, tag="part",
                                             bufs=8, name=f"part_{od}_{st}")
                                nc.scalar.copy(pt[:], psos[st][:])
                                part[(od, st)] = pt
                            else:
                                oso = po.tile([128, 1024], BF, tag="oso",
                                              bufs=2, name=f"oso_{od}_{st}")
                                nc.vector.tensor_tensor(
                                    out=oso[:], in0=part[(od, st)][:],
                                    in1=psos[st][:], op=ADD)
                                nc.gpsimd.dma_start(
                                    out=out_part[128 * st:128 * st + 128,
                                                 ods],
                                    in_=oso[:])

    nc.compile()
    return nc


def _rope_tables():
    inv_freq = 1.0 / (ROPE_BASE ** (np.arange(0, HD, 2, dtype=np.float32) / HD))
    t = np.arange(S, dtype=np.float32)
    freqs = np.outer(t, inv_freq).astype(np.float32)  # [S, 32]
    cosT = np.cos(freqs).T  # [32, S]
    sinT = np.sin(freqs).T
    cos64 = np.concatenate([cosT, cosT], axis=0)          # [64, S]
    sin64 = np.concatenate([-sinT, sinT], axis=0)         # signed
    bf = ml_dtypes.bfloat16
    cos2 = np.concatenate([cos64, cos64], axis=0).astype(bf)
    sin2 = np.concatenate([sin64, sin64], axis=0).astype(bf)
    return cos2, sin2


def _np_reference(hidden_states, attention_mask, q_w, q_b, k_w, k_b, v_w, v_b,
                  o_w):
    hs = hidden_states.astype(np.float64)
    q = hs @ q_w.T.astype(np.float64) + q_b
    k = hs @ k_w.T.astype(np.float64) + k_b
    v = hs @ v_w.T.astype(np.float64) + v_b
    q = q.reshape(B, S, NH, HD).transpose(0, 2, 1, 3)
    k = k.reshape(B, S, NKV, HD).transpose(0, 2, 1, 3)
    v = v.reshape(B, S, NKV, HD).transpose(0, 2, 1, 3)
    inv_freq = 1.0 / (ROPE_BASE ** (np.arange(0, HD, 2) / HD))
    t = np.arange(S)
    freqs = np.outer(t, inv_freq)
    emb = np.concatenate([freqs, freqs], axis=-1)
    cos, sin = np.cos(emb), np.sin(emb)

    def rot(x):
        h = x.shape[-1] // 2
        return np.concatenate([-x[..., h:], x[..., :h]], axis=-1)

    q = q * cos + rot(q) * sin
    k = k * cos + rot(k) * sin
    k = np.repeat(k, GROUPS, axis=1)
    v = np.repeat(v, GROUPS, axis=1)
    sc = np.einsum("bhqd,bhkd->bhqk", q, k) / math.sqrt(HD)
    sc = sc + attention_mask.astype(np.float64)
    sc = sc - sc.max(axis=-1, keepdims=True)
    p = np.exp(sc)
    p = p / p.sum(axis=-1, keepdims=True)
    out = np.einsum("bhqk,bhkd->bhqd", p, v)
    out = out.transpose(0, 2, 1, 3).reshape(B, S, HID)
    return (out @ o_w.T.astype(np.float64)).astype(np.float32)


def _pack_chunks17(mat, bias):
    """[2048, M] weights + [M] bias -> [17, 128, M] with bias in row 0 of
    chunk 16."""
    m = mat.shape[1]
    out = np.zeros((17, 128, m), dtype=mat.dtype)
    out[:16] = mat.reshape(16, 128, m)
    out[16, 0, :] = bias
    return out


def _make_in_maps(inputs):
    hs = np.asarray(inputs["hidden_states"], np.float32)
    q_w = np.asarray(inputs["q_w"], np.float32)
    q_b = np.asarray(inputs["q_b"], np.float32)
    k_w = np.asarray(inputs["k_w"], np.float32)
    k_b = np.asarray(inputs["k_b"], np.float32)
    v_w = np.asarray(inputs["v_w"], np.float32)
    v_b = np.asarray(inputs["v_b"], np.float32)
    o_w = np.asarray(inputs["o_w"], np.float32)
    mask = np.asarray(inputs["attention_mask"], np.float32)
    m2 = mask[0, 0]

    bf = ml_dtypes.bfloat16
    cos2, sin2 = _rope_tables()
    # binary mask for the diagonal block, transposed orientation [k, q]
    maskbin = (m2[0:128, 0:128].T == 0.0).astype(bf)

    # wo rows in A2A arrival order: chunk 8*hf + s = heads (4s+2hf, 4s+2hf+1)
    o_wT = np.ascontiguousarray(o_w.T.astype(bf))  # [2048 in, 2048 out]
    rows = []
    for hf in range(2):
        for s in range(8):
            h = 4 * s + 2 * hf
            rows.extend(range(64 * h, 64 * h + 64))
            rows.extend(range(64 * (h + 1), 64 * (h + 1) + 64))
    wo_np = o_wT[np.array(rows)].reshape(16, 128, HID)

    hsT_packed = np.zeros((2, 17, 128, S), dtype=bf)
    for b in range(B):
        hsT_packed[b, :16] = np.ascontiguousarray(hs[b].T).astype(bf).reshape(
            16, 128, S)
        hsT_packed[b, 16, 0, :] = 1.0

    q_wT = np.ascontiguousarray(q_w.T).astype(bf)  # [2048, 2048]
    k_wT = np.ascontiguousarray(k_w.T).astype(bf)  # [2048, 512]
    v_wT = np.ascontiguousarray(v_w.T).astype(bf)

    in_maps = []
    for c in range(NCORES):
        kv = slice(64 * c, 64 * c + 64)
        kvw0 = np.concatenate([k_wT[:, kv], v_wT[:, kv]], axis=1)
        kvw1 = np.concatenate([v_wT[:, kv], k_wT[:, kv]], axis=1)
        kvb0 = np.concatenate([k_b[kv], v_b[kv]]).astype(bf)
        kvb1 = np.concatenate([v_b[kv], k_b[kv]]).astype(bf)
        in_maps.append({
            "hsT": hsT_packed,
            "wq": _pack_chunks17(q_wT[:, 256 * c:256 * c + 256],
                                 q_b[256 * c:256 * c + 256].astype(bf)),
            "wkv0": _pack_chunks17(kvw0, kvb0),
            "wkv1": _pack_chunks17(kvw1, kvb1),
            "wo": wo_np,
            "cos2": cos2,
            "sin2": sin2,
            "maskbin": maskbin,
        })
    return in_maps


def kernel(**inputs):
    mask = np.asarray(inputs["attention_mask"], np.float32)
    m2 = mask[0, 0]
    causal_ok = bool(
        np.all(m2[np.tril_indices(S)] == 0.0)
        and np.all(m2[np.triu_indices(S, 1)] < -1e8))
    if not causal_ok:
        return _np_reference(
            np.asarray(inputs["hidden_states"], np.float32), mask,
            *(np.asarray(inputs[k], np.float32)
              for k in ("q_w", "q_b", "k_w", "k_b", "v_w", "v_b", "o_w")))

    if "nc" not in _CACHED:
        _CACHED["nc"] = _build_nc()
    nc = _CACHED["nc"]
    in_maps = _make_in_maps(inputs)

    res = run_bass_kernel_spmd(nc, in_maps, list(range(NCORES)))

    out = np.empty((B, S, HID), dtype=np.float32)
    for c in range(NCORES):
        b, g = c // 4, c % 4
        out[b, 512 * g:512 * g + 512, :] = np.asarray(
            res.results[c]["out_part"], np.float32)
    return out


# revision 28
# speedup vs baseline: 1.0161x; 1.0161x over previous
"""GroupedQueryAttention Trainium2 kernel (8 NeuronCores).

Sharding: core c -> q-heads [4c,4c+4), kv-head c, BOTH batches (tensor
parallel 8-way on heads). Output tokens: batch c//4, slice 512*(c%4).

Per core: qkv projection (bias via ones-row 17th contraction chunk) + RoPE;
flash-style causal attention in transposed-score orientation probsT[k,q],
head-PAIRED on the PE: pair p in {0,1} holds heads (2p, 2p+1) in partition
halves, batch b processed as separate super-unit (p,b). The kv head is
shared by both halves, so kT2 is duplicated across partition halves via an
SBUF->SBUF DMA. Denominators via a ones-column appended to V. Exp split:
half 0 -> ScalarE activation, half 1 -> VectorE int16 Schraudolph
(bf16_bits = int16(score*A + B)). Normalization per 512-token PV chunk:
reciprocal_approx_fast of the PSUM den row + gpsimd partition_broadcast +
multiply, sent straight to the AllToAll input (head-split -> token-split;
every A2A byte useful, no blending). A2A#1 (heads 0,1) fires mid-attention;
A2A#2 (heads 2,3) right after the last chunk; o_proj GEMM for the first
head-half hides A2A#2. wo is streamed from DRAM (no SBUF residency).

Self-contained: hardcodes all shapes; only imports the concourse toolchain.
"""

import sys

for _p in ("/opt/trn_rl_repo", "/root/.axon_site/_ro/trn_rl_repo"):
    if _p not in sys.path:
        sys.path.insert(0, _p)

import math

import numpy as np
import ml_dtypes

import concourse.bass as bass
import concourse.mybir as mybir
import concourse.tile as tile
from concourse import bacc
from concourse.bass_utils import run_bass_kernel_spmd
from concourse.masks import make_identity

B, S, HID = 2, 2048, 2048
NH, NKV, HD = 32, 8, 64
GROUPS = NH // NKV
ROPE_BASE = 10000.0
NCORES = 8

BF = mybir.dt.bfloat16
F32 = mybir.dt.float32
I16 = mybir.dt.int16

NB = S // 128  # 16 k strip blocks
LQ = [S - 128 * j for j in range(NB)]
OFFX = [0]
for _j in range(NB):
    OFFX.append(OFFX[-1] + LQ[_j])
TOT = OFFX[-1]  # 17408

LOG2E = 1.4426950408889634
SCALE = 1.0 / math.sqrt(HD)
SCH_A = SCALE * LOG2E * 128.0
SCH_B = 127.0 * 128.0 - 5.6

_CACHED = {}


def _build_nc():
    nc = bacc.Bacc("TRN2", target_bir_lowering=False, debug=False,
                   num_devices=NCORES)

    hsT = nc.declare_dram_parameter("hsT", [2, 17, 128, S], BF, isOutput=False)
    wq = nc.declare_dram_parameter("wq", [17, 128, 256], BF, isOutput=False)
    wkv0 = nc.declare_dram_parameter("wkv0", [17, 128, 128], BF,
                                     isOutput=False)
    wkv1 = nc.declare_dram_parameter("wkv1", [17, 128, 128], BF,
                                     isOutput=False)
    wo = nc.declare_dram_parameter("wo", [16, 128, HID], BF, isOutput=False)
    cos2 = nc.declare_dram_parameter("cos2", [128, S], BF, isOutput=False)
    sin2 = nc.declare_dram_parameter("sin2", [128, S], BF, isOutput=False)
    mb = nc.declare_dram_parameter("maskbin", [128, 128], BF, isOutput=False)
    ones2d = nc.declare_dram_parameter("ones2", [2, 128], BF, isOutput=False)
    out_part = nc.declare_dram_parameter("out_part", [512, HID], BF,
                                         isOutput=True)

    MULT = mybir.AluOpType.mult
    ADD = mybir.AluOpType.add
    EXP = mybir.ActivationFunctionType.Exp

    with tile.TileContext(nc) as tc:
        with tc.tile_pool(name="pers", bufs=1) as pers, \
             tc.tile_pool(name="dram", bufs=1, space="DRAM") as dram:
            # qT2[dim-of-head-pair, pair, batch, pos]; halves = heads 2p/2p+1
            qT2 = pers.tile([128, 2, 2, S], BF)
            # kT2[dup kv dims (both halves identical), batch, pos]
            kT2 = pers.tile([128, 2, S], BF)
            # v_aug[kpos, batch, block, dim+ones]
            v_aug = pers.tile([128, 2, NB, 65], BF)
            idn = pers.tile([128, 128], BF)
            make_identity(nc, idn)
            maskb = pers.tile([128, 128], BF)
            nc.sync.dma_start(out=maskb[:], in_=mb[:])
            nc.vector.memset(v_aug[:, :, :, 64:65], 1.0)
            # block-diagonal ones for the PE denominator broadcast:
            # rb = ones2.T @ [rec_h0; rec_h1] -> rows 0:64 = rec_h0, 64:128 =
            # rec_h1
            ones2 = pers.tile([2, 128], BF)
            nc.sync.dma_start(out=ones2[:], in_=ones2d[:])
            # gathered attention rows for o_proj (one tile per A2A)
            attg0 = pers.tile([128, 8, 512], BF)
            attg1 = pers.tile([128, 8, 512], BF)

            a2aA_in = dram.tile([NCORES, 128, 512], BF)
            a2aA_out = dram.tile([NCORES, 128, 512], BF)
            a2aB_in = dram.tile([NCORES, 128, 512], BF)
            a2aB_out = dram.tile([NCORES, 128, 512], BF)

            # ---------------- qkv projection + RoPE (bf16) ----------------
            with tc.tile_pool(name="proj", bufs=1) as pj, \
                 tc.tile_pool(name="projp", bufs=1, space="PSUM") as pjp:
                hsT_sb = pj.tile([128, 2, 17, S], BF)
                cos_sb = pj.tile([128, S], BF)
                sin_sb = pj.tile([128, S], BF)
                vT2 = pj.tile([128, S], BF)  # rows: [v(b1) | v(b0)]

                # startup loads spread over the 3 DMA trigger queues; order
                # matches consumption (kv(b0) first). wkv0 fully pre-fetched
                # first on gpsimd (bufs=17: no buffer reuse -> no pre-emission
                # race) so the first matmuls aren't queued behind hsT chunks.
                wkv0_sb = []
                for ki in range(17):
                    wt = pj.tile([128, 128], BF, tag="wkv0", bufs=17,
                                 name=f"wkv0_{ki}")
                    nc.gpsimd.dma_start(out=wt[:], in_=wkv0[ki])
                    wkv0_sb.append(wt)
                for ki in range(17):
                    eng = nc.sync if ki % 2 == 0 else nc.gpsimd
                    eng.dma_start(out=hsT_sb[:, 0, ki, :], in_=hsT[0, ki])
                for ki in range(17):
                    nc.scalar.dma_start(out=hsT_sb[:, 1, ki, :],
                                        in_=hsT[1, ki])
                nc.sync.dma_start(out=cos_sb[:], in_=cos2[:])
                nc.sync.dma_start(out=sin_sb[:], in_=sin2[:])

                def rope(pq, lo, hi, dst_slices, sl):
                    """RoPE rows [lo:hi) of psum pq; write to dst slices."""
                    n = hi - lo
                    tmp = pj.tile([128, 512], BF, tag="ropetmp", bufs=3)
                    pairs = [(0, 32), (32, 0), (64, 96), (96, 64)]
                    for (a, bb) in pairs:
                        if a >= n:
                            continue
                        nc.vector.tensor_tensor(
                            out=tmp[lo + a:lo + a + 32, :],
                            in0=pq[lo + bb:lo + bb + 32, :],
                            in1=sin_sb[lo + a:lo + a + 32, sl], op=MULT)
                    tmp2 = pj.tile([128, 512], BF, tag="ropetmp2", bufs=3)
                    nc.vector.tensor_tensor(out=tmp2[lo:hi, :],
                                            in0=pq[lo:hi, :],
                                            in1=cos_sb[lo:hi, sl], op=MULT)
                    for dst in dst_slices:
                        nc.vector.tensor_tensor(out=dst, in0=tmp2[lo:hi, :],
                                                in1=tmp[lo:hi, :], op=ADD)

                # kv units: b0 uses [k|v] weights, b1 uses [v|k]
                for b in range(2):
                    accs = [pjp.tile([128, 512], F32, tag="acc", bufs=6,
                                     name=f"acckv_{b}_{Q}") for Q in range(4)]
                    for ki in range(17):
                        if b == 0:
                            wt = wkv0_sb[ki]
                        else:
                            wt = pj.tile([128, 128], BF, tag="wkv1", bufs=4,
                                         name=f"wkv1_{ki}")
                            nc.gpsimd.dma_start(out=wt[:], in_=wkv1[ki])
                        for Q in range(4):
                            nc.tensor.matmul(
                                accs[Q][:], lhsT=wt[:],
                                rhs=hsT_sb[:, b, ki, 512 * Q:512 * Q + 512],
                                start=(ki == 0), stop=(ki == 16))
                    for Q in range(4):
                        sl = slice(512 * Q, 512 * Q + 512)
                        pq = accs[Q]
                        if b == 0:
                            # rows 0:64 = k(b0) -> kT2[0:64, 0]
                            rope(pq, 0, 64, [kT2[0:64, 0, sl]], sl)
                            nc.scalar.copy(vT2[64:128, sl], pq[64:128, :])
                        else:
                            # rows 64:128 = k(b1) -> kT2[64:128, 1]
                            rope(pq, 64, 128, [kT2[64:128, 1, sl]], sl)
                            nc.scalar.copy(vT2[0:64, sl], pq[0:64, :])
                    # duplicate kv head across partition halves
                    # (partition-shifted DVE copies, 512 cols each)
                    for Q in range(4):
                        sl = slice(512 * Q, 512 * Q + 512)
                        if b == 0:
                            nc.vector.tensor_copy(kT2[64:128, 0, sl],
                                                  kT2[0:64, 0, sl])
                        else:
                            nc.vector.tensor_copy(kT2[0:64, 1, sl],
                                                  kT2[64:128, 1, sl])

                # q units: pair p holds heads (2p, 2p+1) in halves
                for p in range(2):
                    for b in range(2):
                        accs = [pjp.tile([128, 512], F32, tag="acc", bufs=6,
                                         name=f"accq_{p}_{b}_{Q}")
                                for Q in range(4)]
                        for ki in range(17):
                            wt = pj.tile([128, 128], BF, tag="wq", bufs=4,
                                         name=f"wq_{p}_{b}_{ki}")
                            nc.scalar.dma_start(
                                out=wt[:],
                                in_=wq[ki, :, 128 * p:128 * p + 128])
                            for Q in range(4):
                                nc.tensor.matmul(
                                    accs[Q][:], lhsT=wt[:],
                                    rhs=hsT_sb[:, b, ki,
                                               512 * Q:512 * Q + 512],
                                    start=(ki == 0), stop=(ki == 16))
                        for Q in range(4):
                            sl = slice(512 * Q, 512 * Q + 512)
                            rope(accs[Q], 0, 128, [qT2[:, p, b, sl]], sl)

                # v: vT2 rows [v(b1)|v(b0)] -> v_aug[kpos, batch, block, dim]
                for kb in range(NB):
                    pvt = pjp.tile([128, 128], BF, tag="vt", bufs=2,
                                   name=f"pvt_{kb}")
                    nc.tensor.transpose(pvt[:], vT2[:, 128 * kb:128 * kb + 128],
                                        idn[:])
                    nc.vector.tensor_copy(v_aug[:, 1, kb, 0:64], pvt[:, 0:64])
                    nc.vector.tensor_copy(v_aug[:, 0, kb, 0:64],
                                          pvt[:, 64:128])

            # ---------------- attention ----------------
            with tc.tile_pool(name="att", bufs=1) as at, \
                 tc.tile_pool(name="attp", bufs=1, space="PSUM") as atp:

                def qk_strip(p, b, j, probsL, probsH):
                    q0 = 128 * j
                    L = LQ[j]
                    for cb in range(0, L, 512):
                        w = min(512, L - cb)
                        for h, probs in ((0, probsL), (1, probsH)):
                            ps = atp.tile([128, 512], F32, tag="sc", bufs=4,
                                          name=f"sc_{p}_{b}_{j}_{cb}_{h}")
                            nc.tensor.matmul(
                                ps[:, 0:w],
                                lhsT=kT2[64 * h:64 * h + 64, b,
                                         q0:q0 + 128],
                                rhs=qT2[64 * h:64 * h + 64, p, b,
                                        q0 + cb:q0 + cb + w],
                                start=True, stop=True)
                            dst = probs[:, OFFX[j] + cb:OFFX[j] + cb + w]
                            if h == 0:
                                nc.scalar.activation(dst, ps[:, 0:w], EXP,
                                                     scale=SCALE)
                            else:
                                nc.vector.tensor_scalar(
                                    out=dst.bitcast(I16), in0=ps[:, 0:w],
                                    scalar1=SCH_A, scalar2=SCH_B,
                                    op0=MULT, op1=ADD)
                            if cb == 0:
                                # causal mask on the diagonal block
                                nc.vector.tensor_tensor(
                                    out=probs[:, OFFX[j]:OFFX[j] + 128],
                                    in0=probs[:, OFFX[j]:OFFX[j] + 128],
                                    in1=maskb[:], op=MULT)

                def pv_chunk(p, b, h, c, probs, den):
                    """PV for 512-token chunk c; den row h of the pair's den
                    tile is filled; pvt copied to SBUF aa (releases PSUM)."""
                    pvt = atp.tile([65, 512], F32, tag="pv", bufs=2,
                                   name=f"pv_{p}_{b}_{h}_{c}")
                    for j in range(4 * c + 4):
                        if j <= 4 * c:
                            col = OFFX[j] + 512 * c - 128 * j
                            nc.tensor.matmul(
                                pvt[:, 0:512], lhsT=v_aug[:, b, j, :],
                                rhs=probs[:, col:col + 512],
                                start=(j == 0), stop=(j == 4 * c + 3))
                        else:
                            d0 = 128 * (j - 4 * c)
                            nc.tensor.matmul(
                                pvt[:, d0:512], lhsT=v_aug[:, b, j, :],
                                rhs=probs[:, OFFX[j]:OFFX[j] + 512 - d0],
                                start=False, stop=(j == 4 * c + 3))
                    aa = at.tile([64, 512], BF, tag="aa", bufs=4,
                                 name=f"aa_{p}_{b}_{h}_{c}")
                    nc.scalar.copy(aa[:], pvt[0:64, :])
                    nc.scalar.copy(den[0:1, h, :], pvt[64:65, :])
                    return aa

                def norm_send(p, b, c, den, aa0, aa1):
                    """Reciprocal + rank-1 PE broadcast of the denominators,
                    normalize, send to dest core 4b+c."""
                    rec = at.tile([1, 2, 512], F32, tag="rec", bufs=1,
                                  name=f"rec_{p}_{b}_{c}")
                    nc.vector.reciprocal_approx_fast(out=rec[:], in_=den[:])
                    rec_b = at.tile([1, 2, 512], BF, tag="recb", bufs=1,
                                    name=f"recb_{p}_{b}_{c}")
                    nc.vector.tensor_copy(rec_b[:], rec[:])
                    a2a_in = a2aA_in if p == 0 else a2aB_in
                    for h, aa in ((0, aa0), (1, aa1)):
                        rb = atp.tile([64, 512], F32, tag="rb", bufs=2,
                                      name=f"rb_{p}_{b}_{h}_{c}")
                        nc.tensor.matmul(rb[:], lhsT=ones2[0:1, 0:64],
                                         rhs=rec_b[0:1, h, :],
                                         start=True, stop=True)
                        an = at.tile([64, 512], BF, tag="an", bufs=2,
                                     name=f"an_{p}_{b}_{h}_{c}")
                        nc.vector.tensor_tensor(
                            out=an[:], in0=aa[:], in1=rb[:], op=MULT)
                        nc.gpsimd.dma_start(
                            out=a2a_in[4 * b + c, 64 * h:64 * h + 64, :],
                            in_=an[:])

                for p in range(2):
                    for b in range(2):
                        probsL = at.tile([128, TOT], BF, tag="probsL", bufs=2,
                                         name=f"probsL_{p}_{b}")
                        probsH = at.tile([128, TOT], BF, tag="probsH", bufs=2,
                                         name=f"probsH_{p}_{b}")
                        # norm_send deferred 2 strips so the rb matmul never
                        # stalls the in-order PE queue on the recip chain
                        pending = None
                        for j in range(NB):
                            qk_strip(p, b, j, probsL, probsH)
                            if pending is not None and j % 4 == 1:
                                norm_send(*pending)
                                pending = None
                            if j % 4 == 3:
                                c = j // 4
                                den = at.tile([1, 2, 512], F32, tag="den",
                                              bufs=2, name=f"den_{p}_{b}_{c}")
                                aa0 = pv_chunk(p, b, 0, c, probsL, den)
                                aa1 = pv_chunk(p, b, 1, c, probsH, den)
                                pending = (p, b, c, den, aa0, aa1)
                        norm_send(*pending)
                    # pair's 16 sends complete -> AllToAll
                    a2a_in = a2aA_in if p == 0 else a2aB_in
                    a2a_out = a2aA_out if p == 0 else a2aB_out
                    attg = attg0 if p == 0 else attg1
                    nc.gpsimd.collective_compute(
                        "AllToAll", mybir.AluOpType.bypass,
                        replica_groups=[list(range(NCORES))],
                        ins=[a2a_in.opt()], outs=[a2a_out.opt()])
                    # readback on the (idle) sync queue so the A2A wait
                    # doesn't block exp/copy work on other queues
                    nc.sync.dma_start(
                        out=attg[:],
                        in_=a2a_out.rearrange("s p n -> p s n"))

            # ---------------- o_proj (my 512 tokens, all 2048 od) ---------
            # passes: (hf, od-half); wo streamed from DRAM; hf=0 GEMMs hide
            # A2A#2.
            with tc.tile_pool(name="op", bufs=1) as po, \
                 tc.tile_pool(name="opp", bufs=1, space="PSUM") as pop:
                part = {}
                for hf, attg in ((0, attg0), (1, attg1)):
                    for od in range(2):
                        ods = slice(1024 * od, 1024 * od + 1024)
                        psos = [pop.tile([128, 1024], F32, tag="po", bufs=4,
                                         name=f"pso_{hf}_{od}_{st}")
                                for st in range(4)]
                        for k8 in range(8):
                            wot = po.tile([128, 1024], BF, tag="wo", bufs=3,
                                          name=f"wo_{hf}_{od}_{k8}")
                            nc.scalar.dma_start(out=wot[:],
                                                in_=wo[8 * hf + k8, :, ods])
                            for st in range(4):
                                lhsT = attg[:, k8, 128 * st:128 * st + 128]
                                for u in range(2):
                                    nc.tensor.matmul(
                                        psos[st][:, 512 * u:512 * u + 512],
                                        lhsT=lhsT,
                                        rhs=wot[:, 512 * u:512 * u + 512],
                                        start=(k8 == 0), stop=(k8 == 7))
                        for st in range(4):
                            if hf == 0:
                                pt = po.tile([128, 1024], BF, tag="part",
                                             bufs=8, name=f"part_{od}_{st}")
                                nc.scalar.copy(pt[:], psos[st][:])
                                part[(od, st)] = pt
                            else:
                                oso = po.tile([128, 1024], BF, tag="oso",
                                              bufs=2, name=f"oso_{od}_{st}")
                                nc.vector.tensor_tensor(
                                    out=oso[:], in0=part[(od, st)][:],
                                    in1=psos[st][:], op=ADD)
                                nc.gpsimd.dma_start(
                                    out=out_part[128 * st:128 * st + 128,
                                                 ods],
                                    in_=oso[:])

    nc.compile()
    return nc


def _rope_tables():
    inv_freq = 1.0 / (ROPE_BASE ** (np.arange(0, HD, 2, dtype=np.float32) / HD))
    t = np.arange(S, dtype=np.float32)
    freqs = np.outer(t, inv_freq).astype(np.float32)  # [S, 32]
    cosT = np.cos(freqs).T  # [32, S]
    sinT = np.sin(freqs).T
    cos64 = np.concatenate([cosT, cosT], axis=0)          # [64, S]
    sin64 = np.concatenate([-sinT, sinT], axis=0)         # signed
    bf = ml_dtypes.bfloat16
    cos2 = np.concatenate([cos64, cos64], axis=0).astype(bf)
    sin2 = np.concatenate([sin64, sin64], axis=0).astype(bf)
    return cos2, sin2


def _np_reference(hidden_states, attention_mask, q_w, q_b, k_w, k_b, v_w, v_b,
                  o_w):
    hs = hidden_states.astype(np.float64)
    q = hs @ q_w.T.astype(np.float64) + q_b
    k = hs @ k_w.T.astype(np.float64) + k_b
    v = hs @ v_w.T.astype(np.float64) + v_b
    q = q.reshape(B, S, NH, HD).transpose(0, 2, 1, 3)
    k = k.reshape(B, S, NKV, HD).transpose(0, 2, 1, 3)
    v = v.reshape(B, S, NKV, HD).transpose(0, 2, 1, 3)
    inv_freq = 1.0 / (ROPE_BASE ** (np.arange(0, HD, 2) / HD))
    t = np.arange(S)
    freqs = np.outer(t, inv_freq)
    emb = np.concatenate([freqs, freqs], axis=-1)
    cos, sin = np.cos(emb), np.sin(emb)

    def rot(x):
        h = x.shape[-1] // 2
        return np.concatenate([-x[..., h:], x[..., :h]], axis=-1)

    q = q * cos + rot(q) * sin
    k = k * cos + rot(k) * sin
    k = np.repeat(k, GROUPS, axis=1)
    v = np.repeat(v, GROUPS, axis=1)
    sc = np.einsum("bhqd,bhkd->bhqk", q, k) / math.sqrt(HD)
    sc = sc + attention_mask.astype(np.float64)
    sc = sc - sc.max(axis=-1, keepdims=True)
    p = np.exp(sc)
    p = p / p.sum(axis=-1, keepdims=True)
    out = np.einsum("bhqk,bhkd->bhqd", p, v)
    out = out.transpose(0, 2, 1, 3).reshape(B, S, HID)
    return (out @ o_w.T.astype(np.float64)).astype(np.float32)


def _pack_chunks17(mat, bias):
    """[2048, M] weights + [M] bias -> [17, 128, M] with bias in row 0 of
    chunk 16."""
    m = mat.shape[1]
    out = np.zeros((17, 128, m), dtype=mat.dtype)
    out[:16] = mat.reshape(16, 128, m)
    out[16, 0, :] = bias
    return out


def _make_in_maps(inputs):
    hs = np.asarray(inputs["hidden_states"], np.float32)
    q_w = np.asarray(inputs["q_w"], np.float32)
    q_b = np.asarray(inputs["q_b"], np.float32)
    k_w = np.asarray(inputs["k_w"], np.float32)
    k_b = np.asarray(inputs["k_b"], np.float32)
    v_w = np.asarray(inputs["v_w"], np.float32)
    v_b = np.asarray(inputs["v_b"], np.float32)
    o_w = np.asarray(inputs["o_w"], np.float32)
    mask = np.asarray(inputs["attention_mask"], np.float32)
    m2 = mask[0, 0]

    bf = ml_dtypes.bfloat16
    cos2, sin2 = _rope_tables()
    # binary mask for the diagonal block, transposed orientation [k, q]
    maskbin = (m2[0:128, 0:128].T == 0.0).astype(bf)

    # wo rows in A2A arrival order: chunk 8*hf + s = heads (4s+2hf, 4s+2hf+1)
    o_wT = np.ascontiguousarray(o_w.T.astype(bf))  # [2048 in, 2048 out]
    rows = []
    for hf in range(2):
        for s in range(8):
            h = 4 * s + 2 * hf
            rows.extend(range(64 * h, 64 * h + 64))
            rows.extend(range(64 * (h + 1), 64 * (h + 1) + 64))
    wo_np = o_wT[np.array(rows)].reshape(16, 128, HID)

    hsT_packed = np.zeros((2, 17, 128, S), dtype=bf)
    for b in range(B):
        hsT_packed[b, :16] = np.ascontiguousarray(hs[b].T).astype(bf).reshape(
            16, 128, S)
        hsT_packed[b, 16, 0, :] = 1.0

    q_wT = np.ascontiguousarray(q_w.T).astype(bf)  # [2048, 2048]
    k_wT = np.ascontiguousarray(k_w.T).astype(bf)  # [2048, 512]
    v_wT = np.ascontiguousarray(v_w.T).astype(bf)

    # block-diagonal broadcast helper: rb = ones2.T @ [rec0; rec1]
    ones2_np = np.zeros((2, 128), dtype=bf)
    ones2_np[0, 0:64] = 1.0
    ones2_np[1, 64:128] = 1.0

    in_maps = []
    for c in range(NCORES):
        kv = slice(64 * c, 64 * c + 64)
        kvw0 = np.concatenate([k_wT[:, kv], v_wT[:, kv]], axis=1)
        kvw1 = np.concatenate([v_wT[:, kv], k_wT[:, kv]], axis=1)
        kvb0 = np.concatenate([k_b[kv], v_b[kv]]).astype(bf)
        kvb1 = np.concatenate([v_b[kv], k_b[kv]]).astype(bf)
        in_maps.append({
            "hsT": hsT_packed,
            "wq": _pack_chunks17(q_wT[:, 256 * c:256 * c + 256],
                                 q_b[256 * c:256 * c + 256].astype(bf)),
            "wkv0": _pack_chunks17(kvw0, kvb0),
            "wkv1": _pack_chunks17(kvw1, kvb1),
            "wo": wo_np,
            "cos2": cos2,
            "sin2": sin2,
            "maskbin": maskbin,
            "ones2": ones2_np,
        })
    return in_maps


def kernel(**inputs):
    mask = np.asarray(inputs["attention_mask"], np.float32)
    m2 = mask[0, 0]
    causal_ok = bool(
        np.all(m2[np.tril_indices(S)] == 0.0)
        and np.all(m2[np.triu_indices(S, 1)] < -1e8))
    if not causal_ok:
        return _np_reference(
            np.asarray(inputs["hidden_states"], np.float32), mask,
            *(np.asarray(inputs[k], np.float32)
              for k in ("q_w", "q_b", "k_w", "k_b", "v_w", "v_b", "o_w")))

    if "nc" not in _CACHED:
        _CACHED["nc"] = _build_nc()
    nc = _CACHED["nc"]
    in_maps = _make_in_maps(inputs)

    res = run_bass_kernel_spmd(nc, in_maps, list(range(NCORES)))

    out = np.empty((B, S, HID), dtype=np.float32)
    for c in range(NCORES):
        b, g = c // 4, c % 4
        out[b, 512 * g:512 * g + 512, :] = np.asarray(
            res.results[c]["out_part"], np.float32)
    return out


# revision 32
# speedup vs baseline: 1.0904x; 1.0731x over previous
"""GroupedQueryAttention Trainium2 kernel (8 NeuronCores).

Sharding: core c -> q-heads [4c,4c+4), kv-head c, BOTH batches (tensor
parallel 8-way on heads). Output tokens: batch c//4, slice 512*(c%4).

Per core: qkv projection (bias via ones-row 17th contraction chunk) + RoPE;
flash-style causal attention in transposed-score orientation probsT[k,q],
head-PAIRED on the PE: pair p in {0,1} holds heads (2p, 2p+1) in partition
halves, batch b processed as separate super-unit (p,b). The kv head is
shared by both halves, so kT2 is duplicated across partition halves via an
SBUF->SBUF DMA. Denominators via a ones-column appended to V. Exp split:
half 0 -> ScalarE activation, half 1 -> VectorE int16 Schraudolph
(bf16_bits = int16(score*A + B)). Normalization per 512-token PV chunk:
reciprocal_approx_fast of the PSUM den row + gpsimd partition_broadcast +
multiply, sent straight to the AllToAll input (head-split -> token-split;
every A2A byte useful, no blending). A2A#1 (heads 0,1) fires mid-attention;
A2A#2 (heads 2,3) right after the last chunk; o_proj GEMM for the first
head-half hides A2A#2. wo is streamed from DRAM (no SBUF residency).

Self-contained: hardcodes all shapes; only imports the concourse toolchain.
"""

import sys

for _p in ("/opt/trn_rl_repo", "/root/.axon_site/_ro/trn_rl_repo"):
    if _p not in sys.path:
        sys.path.insert(0, _p)

import math

import numpy as np
import ml_dtypes

import concourse.bass as bass
import concourse.mybir as mybir
import concourse.tile as tile
from concourse import bacc
from concourse.bass_utils import run_bass_kernel_spmd
from concourse.masks import make_identity

B, S, HID = 2, 2048, 2048
NH, NKV, HD = 32, 8, 64
GROUPS = NH // NKV
ROPE_BASE = 10000.0
NCORES = 8

BF = mybir.dt.bfloat16
F32 = mybir.dt.float32
I16 = mybir.dt.int16

NB = S // 128  # 16 k strip blocks
LQ = [S - 128 * j for j in range(NB)]
OFFX = [0]
for _j in range(NB):
    OFFX.append(OFFX[-1] + LQ[_j])
TOT = OFFX[-1]  # 17408

LOG2E = 1.4426950408889634
SCALE = 1.0 / math.sqrt(HD)
SCH_A = SCALE * LOG2E * 128.0
SCH_B = 127.0 * 128.0 - 5.6

_CACHED = {}


def _build_nc():
    nc = bacc.Bacc("TRN2", target_bir_lowering=False, debug=False,
                   num_devices=NCORES)

    hsT = nc.declare_dram_parameter("hsT", [2, 17, 128, S], BF, isOutput=False)
    wq = nc.declare_dram_parameter("wq", [17, 128, 256], BF, isOutput=False)
    wkv0 = nc.declare_dram_parameter("wkv0", [17, 128, 128], BF,
                                     isOutput=False)
    wkv1 = nc.declare_dram_parameter("wkv1", [17, 128, 128], BF,
                                     isOutput=False)
    wo = nc.declare_dram_parameter("wo", [16, 128, HID], BF, isOutput=False)
    cos2 = nc.declare_dram_parameter("cos2", [128, S], BF, isOutput=False)
    sin2 = nc.declare_dram_parameter("sin2", [128, S], BF, isOutput=False)
    mb = nc.declare_dram_parameter("maskbin", [128, 128], BF, isOutput=False)
    ones2d = nc.declare_dram_parameter("ones2", [2, 128], BF, isOutput=False)
    out_part = nc.declare_dram_parameter("out_part", [512, HID], BF,
                                         isOutput=True)

    MULT = mybir.AluOpType.mult
    ADD = mybir.AluOpType.add
    EXP = mybir.ActivationFunctionType.Exp

    with tile.TileContext(nc) as tc:
        with tc.tile_pool(name="pers", bufs=1) as pers, \
             tc.tile_pool(name="dram", bufs=1, space="DRAM") as dram:
            # qT2[dim-of-head-pair, pair, batch, pos]; halves = heads 2p/2p+1
            qT2 = pers.tile([128, 2, 2, S], BF)
            # kT2[dup kv dims (both halves identical), batch, pos]
            kT2 = pers.tile([128, 2, S], BF)
            # v_aug[kpos, batch, block, dim+ones]
            v_aug = pers.tile([128, 2, NB, 65], BF)
            idn = pers.tile([128, 128], BF)
            make_identity(nc, idn)
            maskb = pers.tile([128, 128], BF)
            nc.sync.dma_start(out=maskb[:], in_=mb[:])
            nc.vector.memset(v_aug[:, :, :, 64:65], 1.0)
            # block-diagonal ones for the PE denominator broadcast:
            # rb = ones2.T @ [rec_h0; rec_h1] -> rows 0:64 = rec_h0, 64:128 =
            # rec_h1
            ones2 = pers.tile([2, 128], BF)
            nc.sync.dma_start(out=ones2[:], in_=ones2d[:])
            # gathered attention rows for o_proj (one tile per A2A)
            attg0 = pers.tile([128, 8, 512], BF)
            attg1 = pers.tile([128, 8, 512], BF)

            a2aA_in = dram.tile([NCORES, 128, 512], BF)
            a2aA_out = dram.tile([NCORES, 128, 512], BF)
            a2aB_in = dram.tile([NCORES, 128, 512], BF)
            a2aB_out = dram.tile([NCORES, 128, 512], BF)

            # ---------------- qkv projection + RoPE (bf16) ----------------
            with tc.tile_pool(name="proj", bufs=1) as pj, \
                 tc.tile_pool(name="projp", bufs=1, space="PSUM") as pjp:
                hsT_sb = pj.tile([128, 2, 17, S], BF)
                cos_sb = pj.tile([128, S], BF)
                sin_sb = pj.tile([128, S], BF)
                vT2 = pj.tile([128, S], BF)  # rows: [v(b1) | v(b0)]

                # startup loads spread over the 3 DMA trigger queues; order
                # matches consumption (kv(b0) first). wkv0 fully pre-fetched
                # first on gpsimd (bufs=17: no buffer reuse -> no pre-emission
                # race) so the first matmuls aren't queued behind hsT chunks.
                wkv0_sb = []
                for ki in range(17):
                    wt = pj.tile([128, 128], BF, tag="wkv0", bufs=17,
                                 name=f"wkv0_{ki}")
                    nc.gpsimd.dma_start(out=wt[:], in_=wkv0[ki])
                    wkv0_sb.append(wt)
                for ki in range(17):
                    eng = nc.sync if ki % 2 == 0 else nc.gpsimd
                    eng.dma_start(out=hsT_sb[:, 0, ki, :], in_=hsT[0, ki])
                for ki in range(17):
                    nc.scalar.dma_start(out=hsT_sb[:, 1, ki, :],
                                        in_=hsT[1, ki])
                nc.sync.dma_start(out=cos_sb[:], in_=cos2[:])
                nc.sync.dma_start(out=sin_sb[:], in_=sin2[:])

                def rope(pq, lo, hi, dst_slices, sl):
                    """RoPE rows [lo:hi) of psum pq; write to dst slices."""
                    n = hi - lo
                    tmp = pj.tile([128, 512], BF, tag="ropetmp", bufs=2)
                    pairs = [(0, 32), (32, 0), (64, 96), (96, 64)]
                    for (a, bb) in pairs:
                        if a >= n:
                            continue
                        nc.vector.tensor_tensor(
                            out=tmp[lo + a:lo + a + 32, :],
                            in0=pq[lo + bb:lo + bb + 32, :],
                            in1=sin_sb[lo + a:lo + a + 32, sl], op=MULT)
                    tmp2 = pj.tile([128, 512], BF, tag="ropetmp2", bufs=2)
                    nc.vector.tensor_tensor(out=tmp2[lo:hi, :],
                                            in0=pq[lo:hi, :],
                                            in1=cos_sb[lo:hi, sl], op=MULT)
                    for dst in dst_slices:
                        nc.vector.tensor_tensor(out=dst, in0=tmp2[lo:hi, :],
                                                in1=tmp[lo:hi, :], op=ADD)

                # kv units: b0 uses [k|v] weights, b1 uses [v|k]
                for b in range(2):
                    accs = [pjp.tile([128, 512], F32, tag="acc", bufs=6,
                                     name=f"acckv_{b}_{Q}") for Q in range(4)]
                    for ki in range(17):
                        if b == 0:
                            wt = wkv0_sb[ki]
                        else:
                            wt = pj.tile([128, 128], BF, tag="wkv1", bufs=4,
                                         name=f"wkv1_{ki}")
                            nc.gpsimd.dma_start(out=wt[:], in_=wkv1[ki])
                        for Q in range(4):
                            nc.tensor.matmul(
                                accs[Q][:], lhsT=wt[:],
                                rhs=hsT_sb[:, b, ki, 512 * Q:512 * Q + 512],
                                start=(ki == 0), stop=(ki == 16))
                    for Q in range(4):
                        sl = slice(512 * Q, 512 * Q + 512)
                        pq = accs[Q]
                        if b == 0:
                            # rows 0:64 = k(b0) -> kT2[0:64, 0]
                            rope(pq, 0, 64, [kT2[0:64, 0, sl]], sl)
                            nc.scalar.copy(vT2[64:128, sl], pq[64:128, :])
                        else:
                            # rows 64:128 = k(b1) -> kT2[64:128, 1]
                            rope(pq, 64, 128, [kT2[64:128, 1, sl]], sl)
                            nc.scalar.copy(vT2[0:64, sl], pq[0:64, :])
                    # duplicate kv head across partition halves
                    # (partition-shifted DVE copies, 512 cols each)
                    for Q in range(4):
                        sl = slice(512 * Q, 512 * Q + 512)
                        if b == 0:
                            nc.vector.tensor_copy(kT2[64:128, 0, sl],
                                                  kT2[0:64, 0, sl])
                        else:
                            nc.vector.tensor_copy(kT2[0:64, 1, sl],
                                                  kT2[64:128, 1, sl])

                # q units: pair p holds heads (2p, 2p+1) in halves.
                # Per-Q ki loops so rope(Q) overlaps the next Q's matmuls
                # instead of piling up at the unit end (hsT is fully
                # resident by the time q units run).
                wq_sb = {}
                for p in range(2):
                    for b in range(2):
                        for Q in range(4):
                            acc = pjp.tile([128, 512], F32, tag="acc",
                                           bufs=6, name=f"accq_{p}_{b}_{Q}")
                            for ki in range(17):
                                if (p, ki) not in wq_sb:
                                    wt = pj.tile([128, 128], BF, tag="wq",
                                                 bufs=17,
                                                 name=f"wq_{p}_{ki}")
                                    nc.scalar.dma_start(
                                        out=wt[:],
                                        in_=wq[ki, :, 128 * p:128 * p + 128])
                                    wq_sb[(p, ki)] = wt
                                nc.tensor.matmul(
                                    acc[:], lhsT=wq_sb[(p, ki)][:],
                                    rhs=hsT_sb[:, b, ki,
                                               512 * Q:512 * Q + 512],
                                    start=(ki == 0), stop=(ki == 16))
                            sl = slice(512 * Q, 512 * Q + 512)
                            rope(acc, 0, 128, [qT2[:, p, b, sl]], sl)

                # v: vT2 rows [v(b1)|v(b0)] -> v_aug[kpos, batch, block, dim]
                for kb in range(NB):
                    pvt = pjp.tile([128, 128], BF, tag="vt", bufs=2,
                                   name=f"pvt_{kb}")
                    nc.tensor.transpose(pvt[:], vT2[:, 128 * kb:128 * kb + 128],
                                        idn[:])
                    nc.vector.tensor_copy(v_aug[:, 1, kb, 0:64], pvt[:, 0:64])
                    nc.vector.tensor_copy(v_aug[:, 0, kb, 0:64],
                                          pvt[:, 64:128])

            # ---------------- attention ----------------
            with tc.tile_pool(name="att", bufs=1) as at, \
                 tc.tile_pool(name="attp", bufs=1, space="PSUM") as atp:

                def qk_strip(p, b, j, probsL, probsH):
                    q0 = 128 * j
                    L = LQ[j]
                    for cb in range(0, L, 512):
                        w = min(512, L - cb)
                        for h, probs in ((0, probsL), (1, probsH)):
                            ps = atp.tile([128, 512], F32, tag="sc", bufs=6,
                                          name=f"sc_{p}_{b}_{j}_{cb}_{h}")
                            nc.tensor.matmul(
                                ps[:, 0:w],
                                lhsT=kT2[64 * h:64 * h + 64, b,
                                         q0:q0 + 128],
                                rhs=qT2[64 * h:64 * h + 64, p, b,
                                        q0 + cb:q0 + cb + w],
                                start=True, stop=True)
                            dst = probs[:, OFFX[j] + cb:OFFX[j] + cb + w]
                            # engine split balances Act vs DVE load
                            if h == 0 or j in (8, 13, 14, 15):
                                nc.scalar.activation(dst, ps[:, 0:w], EXP,
                                                     scale=SCALE)
                            else:
                                nc.vector.tensor_scalar(
                                    out=dst.bitcast(I16), in0=ps[:, 0:w],
                                    scalar1=SCH_A, scalar2=SCH_B,
                                    op0=MULT, op1=ADD)
                            if cb == 0:
                                # causal mask on the diagonal block
                                nc.vector.tensor_tensor(
                                    out=probs[:, OFFX[j]:OFFX[j] + 128],
                                    in0=probs[:, OFFX[j]:OFFX[j] + 128],
                                    in1=maskb[:], op=MULT)

                def pv_chunk(p, b, h, c, probs, den):
                    """PV for 512-token chunk c; den row h of the pair's den
                    tile is filled; pvt copied to SBUF aa (releases PSUM)."""
                    pvt = atp.tile([65, 512], F32, tag="pv", bufs=2,
                                   name=f"pv_{p}_{b}_{h}_{c}")
                    for j in range(4 * c + 4):
                        if j <= 4 * c:
                            col = OFFX[j] + 512 * c - 128 * j
                            nc.tensor.matmul(
                                pvt[:, 0:512], lhsT=v_aug[:, b, j, :],
                                rhs=probs[:, col:col + 512],
                                start=(j == 0), stop=(j == 4 * c + 3))
                        else:
                            d0 = 128 * (j - 4 * c)
                            nc.tensor.matmul(
                                pvt[:, d0:512], lhsT=v_aug[:, b, j, :],
                                rhs=probs[:, OFFX[j]:OFFX[j] + 512 - d0],
                                start=False, stop=(j == 4 * c + 3))
                    aa = at.tile([64, 512], BF, tag="aa", bufs=4,
                                 name=f"aa_{p}_{b}_{h}_{c}")
                    nc.scalar.copy(aa[:], pvt[0:64, :])
                    nc.scalar.copy(den[0:1, h, :], pvt[64:65, :])
                    return aa

                def norm_send(p, b, c, den, aa0, aa1):
                    """Reciprocal + rank-1 PE broadcast of the denominators,
                    normalize, send to dest core 4b+c."""
                    rec = at.tile([1, 2, 512], F32, tag="rec", bufs=1,
                                  name=f"rec_{p}_{b}_{c}")
                    nc.vector.reciprocal_approx_fast(out=rec[:], in_=den[:])
                    rec_b = at.tile([1, 2, 512], BF, tag="recb", bufs=1,
                                    name=f"recb_{p}_{b}_{c}")
                    nc.scalar.copy(rec_b[:], rec[:])
                    a2a_in = a2aA_in if p == 0 else a2aB_in
                    for h, aa in ((0, aa0), (1, aa1)):
                        rb = atp.tile([64, 512], F32, tag="sc", bufs=6,
                                      name=f"rb_{p}_{b}_{h}_{c}")
                        nc.tensor.matmul(rb[:], lhsT=ones2[0:1, 0:64],
                                         rhs=rec_b[0:1, h, :],
                                         start=True, stop=True)
                        an = at.tile([64, 512], BF, tag="an", bufs=2,
                                     name=f"an_{p}_{b}_{h}_{c}")
                        nc.vector.tensor_tensor(
                            out=an[:], in0=aa[:], in1=rb[:], op=MULT)
                        nc.gpsimd.dma_start(
                            out=a2a_in[4 * b + c, 64 * h:64 * h + 64, :],
                            in_=an[:])

                for p in range(2):
                    for b in range(2):
                        probsL = at.tile([128, TOT], BF, tag="probsL", bufs=2,
                                         name=f"probsL_{p}_{b}")
                        probsH = at.tile([128, TOT], BF, tag="probsH", bufs=2,
                                         name=f"probsH_{p}_{b}")
                        # norm_send deferred 2 strips so the rb matmul never
                        # stalls the in-order PE queue on the recip chain
                        pending = None
                        for j in range(NB):
                            qk_strip(p, b, j, probsL, probsH)
                            if pending is not None and j % 4 == 1:
                                norm_send(*pending)
                                pending = None
                            if j % 4 == 3:
                                c = j // 4
                                den = at.tile([1, 2, 512], F32, tag="den",
                                              bufs=2, name=f"den_{p}_{b}_{c}")
                                aa0 = pv_chunk(p, b, 0, c, probsL, den)
                                aa1 = pv_chunk(p, b, 1, c, probsH, den)
                                pending = (p, b, c, den, aa0, aa1)
                        norm_send(*pending)
                    # pair's 16 sends complete -> AllToAll
                    a2a_in = a2aA_in if p == 0 else a2aB_in
                    a2a_out = a2aA_out if p == 0 else a2aB_out
                    attg = attg0 if p == 0 else attg1
                    nc.gpsimd.collective_compute(
                        "AllToAll", mybir.AluOpType.bypass,
                        replica_groups=[list(range(NCORES))],
                        ins=[a2a_in.opt()], outs=[a2a_out.opt()])
                    # readback on the (idle) sync queue so the A2A wait
                    # doesn't block exp/copy work on other queues
                    nc.sync.dma_start(
                        out=attg[:],
                        in_=a2a_out.rearrange("s p n -> p s n"))

            # ---------------- o_proj (my 512 tokens, all 2048 od) ---------
            # passes: (hf, od-half); wo streamed from DRAM; hf=0 GEMMs hide
            # A2A#2.
            with tc.tile_pool(name="op", bufs=1) as po, \
                 tc.tile_pool(name="opp", bufs=1, space="PSUM") as pop:
                part = {}
                for hf, attg in ((0, attg0), (1, attg1)):
                    for od in range(2):
                        ods = slice(1024 * od, 1024 * od + 1024)
                        psos = [pop.tile([128, 1024], F32, tag="po", bufs=4,
                                         name=f"pso_{hf}_{od}_{st}")
                                for st in range(4)]
                        for k8 in range(8):
                            wot = po.tile([128, 1024], BF, tag="wo", bufs=4,
                                          name=f"wo_{hf}_{od}_{k8}")
                            nc.scalar.dma_start(out=wot[:],
                                                in_=wo[8 * hf + k8, :, ods])
                            for st in range(4):
                                lhsT = attg[:, k8, 128 * st:128 * st + 128]
                                for u in range(2):
                                    nc.tensor.matmul(
                                        psos[st][:, 512 * u:512 * u + 512],
                                        lhsT=lhsT,
                                        rhs=wot[:, 512 * u:512 * u + 512],
                                        start=(k8 == 0), stop=(k8 == 7))
                        for st in range(4):
                            if hf == 0:
                                pt = po.tile([128, 1024], BF, tag="part",
                                             bufs=8, name=f"part_{od}_{st}")
                                nc.scalar.copy(pt[:], psos[st][:])
                                part[(od, st)] = pt
                            else:
                                oso = po.tile([128, 1024], BF, tag="oso",
                                              bufs=2, name=f"oso_{od}_{st}")
                                nc.vector.tensor_tensor(
                                    out=oso[:], in0=part[(od, st)][:],
                                    in1=psos[st][:], op=ADD)
                                nc.gpsimd.dma_start(
                                    out=out_part[128 * st:128 * st + 128,
                                                 ods],
                                    in_=oso[:])

    nc.compile()
    return nc


def _rope_tables():
    inv_freq = 1.0 / (ROPE_BASE ** (np.arange(0, HD, 2, dtype=np.float32) / HD))
    t = np.arange(S, dtype=np.float32)
    freqs = np.outer(t, inv_freq).astype(np.float32)  # [S, 32]
    cosT = np.cos(freqs).T  # [32, S]
    sinT = np.sin(freqs).T
    cos64 = np.concatenate([cosT, cosT], axis=0)          # [64, S]
    sin64 = np.concatenate([-sinT, sinT], axis=0)         # signed
    bf = ml_dtypes.bfloat16
    cos2 = np.concatenate([cos64, cos64], axis=0).astype(bf)
    sin2 = np.concatenate([sin64, sin64], axis=0).astype(bf)
    return cos2, sin2


def _np_reference(hidden_states, attention_mask, q_w, q_b, k_w, k_b, v_w, v_b,
                  o_w):
    hs = hidden_states.astype(np.float64)
    q = hs @ q_w.T.astype(np.float64) + q_b
    k = hs @ k_w.T.astype(np.float64) + k_b
    v = hs @ v_w.T.astype(np.float64) + v_b
    q = q.reshape(B, S, NH, HD).transpose(0, 2, 1, 3)
    k = k.reshape(B, S, NKV, HD).transpose(0, 2, 1, 3)
    v = v.reshape(B, S, NKV, HD).transpose(0, 2, 1, 3)
    inv_freq = 1.0 / (ROPE_BASE ** (np.arange(0, HD, 2) / HD))
    t = np.arange(S)
    freqs = np.outer(t, inv_freq)
    emb = np.concatenate([freqs, freqs], axis=-1)
    cos, sin = np.cos(emb), np.sin(emb)

    def rot(x):
        h = x.shape[-1] // 2
        return np.concatenate([-x[..., h:], x[..., :h]], axis=-1)

    q = q * cos + rot(q) * sin
    k = k * cos + rot(k) * sin
    k = np.repeat(k, GROUPS, axis=1)
    v = np.repeat(v, GROUPS, axis=1)
    sc = np.einsum("bhqd,bhkd->bhqk", q, k) / math.sqrt(HD)
    sc = sc + attention_mask.astype(np.float64)
    sc = sc - sc.max(axis=-1, keepdims=True)
    p = np.exp(sc)
    p = p / p.sum(axis=-1, keepdims=True)
    out = np.einsum("bhqk,bhkd->bhqd", p, v)
    out = out.transpose(0, 2, 1, 3).reshape(B, S, HID)
    return (out @ o_w.T.astype(np.float64)).astype(np.float32)


def _pack_chunks17(mat, bias):
    """[2048, M] weights + [M] bias -> [17, 128, M] with bias in row 0 of
    chunk 16."""
    m = mat.shape[1]
    out = np.zeros((17, 128, m), dtype=mat.dtype)
    out[:16] = mat.reshape(16, 128, m)
    out[16, 0, :] = bias
    return out


def _make_in_maps(inputs):
    hs = np.asarray(inputs["hidden_states"], np.float32)
    q_w = np.asarray(inputs["q_w"], np.float32)
    q_b = np.asarray(inputs["q_b"], np.float32)
    k_w = np.asarray(inputs["k_w"], np.float32)
    k_b = np.asarray(inputs["k_b"], np.float32)
    v_w = np.asarray(inputs["v_w"], np.float32)
    v_b = np.asarray(inputs["v_b"], np.float32)
    o_w = np.asarray(inputs["o_w"], np.float32)
    mask = np.asarray(inputs["attention_mask"], np.float32)
    m2 = mask[0, 0]

    bf = ml_dtypes.bfloat16
    cos2, sin2 = _rope_tables()
    # binary mask for the diagonal block, transposed orientation [k, q]
    maskbin = (m2[0:128, 0:128].T == 0.0).astype(bf)

    # wo rows in A2A arrival order: chunk 8*hf + s = heads (4s+2hf, 4s+2hf+1)
    o_wT = np.ascontiguousarray(o_w.T.astype(bf))  # [2048 in, 2048 out]
    rows = []
    for hf in range(2):
        for s in range(8):
            h = 4 * s + 2 * hf
            rows.extend(range(64 * h, 64 * h + 64))
            rows.extend(range(64 * (h + 1), 64 * (h + 1) + 64))
    wo_np = o_wT[np.array(rows)].reshape(16, 128, HID)

    hsT_packed = np.zeros((2, 17, 128, S), dtype=bf)
    for b in range(B):
        hsT_packed[b, :16] = np.ascontiguousarray(hs[b].T).astype(bf).reshape(
            16, 128, S)
        hsT_packed[b, 16, 0, :] = 1.0

    q_wT = np.ascontiguousarray(q_w.T).astype(bf)  # [2048, 2048]
    k_wT = np.ascontiguousarray(k_w.T).astype(bf)  # [2048, 512]
    v_wT = np.ascontiguousarray(v_w.T).astype(bf)

    # block-diagonal broadcast helper: rb = ones2.T @ [rec0; rec1]
    ones2_np = np.zeros((2, 128), dtype=bf)
    ones2_np[0, 0:64] = 1.0
    ones2_np[1, 64:128] = 1.0

    in_maps = []
    for c in range(NCORES):
        kv = slice(64 * c, 64 * c + 64)
        kvw0 = np.concatenate([k_wT[:, kv], v_wT[:, kv]], axis=1)
        kvw1 = np.concatenate([v_wT[:, kv], k_wT[:, kv]], axis=1)
        kvb0 = np.concatenate([k_b[kv], v_b[kv]]).astype(bf)
        kvb1 = np.concatenate([v_b[kv], k_b[kv]]).astype(bf)
        in_maps.append({
            "hsT": hsT_packed,
            "wq": _pack_chunks17(q_wT[:, 256 * c:256 * c + 256],
                                 q_b[256 * c:256 * c + 256].astype(bf)),
            "wkv0": _pack_chunks17(kvw0, kvb0),
            "wkv1": _pack_chunks17(kvw1, kvb1),
            "wo": wo_np,
            "cos2": cos2,
            "sin2": sin2,
            "maskbin": maskbin,
            "ones2": ones2_np,
        })
    return in_maps


def kernel(**inputs):
    mask = np.asarray(inputs["attention_mask"], np.float32)
    m2 = mask[0, 0]
    causal_ok = bool(
        np.all(m2[np.tril_indices(S)] == 0.0)
        and np.all(m2[np.triu_indices(S, 1)] < -1e8))
    if not causal_ok:
        return _np_reference(
            np.asarray(inputs["hidden_states"], np.float32), mask,
            *(np.asarray(inputs[k], np.float32)
              for k in ("q_w", "q_b", "k_w", "k_b", "v_w", "v_b", "o_w")))

    if "nc" not in _CACHED:
        _CACHED["nc"] = _build_nc()
    nc = _CACHED["nc"]
    in_maps = _make_in_maps(inputs)

    res = run_bass_kernel_spmd(nc, in_maps, list(range(NCORES)))

    out = np.empty((B, S, HID), dtype=np.float32)
    for c in range(NCORES):
        b, g = c // 4, c % 4
        out[b, 512 * g:512 * g + 512, :] = np.asarray(
            res.results[c]["out_part"], np.float32)
    return out


# revision 34
# speedup vs baseline: 1.1232x; 1.0301x over previous
"""GroupedQueryAttention Trainium2 kernel (8 NeuronCores).

Sharding: core c -> q-heads [4c,4c+4), kv-head c, BOTH batches (tensor
parallel 8-way on heads). Output tokens: batch c//4, slice 512*(c%4).

Per core: qkv projection (bias via ones-row 17th contraction chunk) + RoPE;
flash-style causal attention in transposed-score orientation probsT[k,q],
head-PAIRED on the PE: pair p in {0,1} holds heads (2p, 2p+1) in partition
halves, batch b processed as separate super-unit (p,b). The kv head is
shared by both halves, so kT2 is duplicated across partition halves via an
SBUF->SBUF DMA. Denominators via a ones-column appended to V. Exp split:
half 0 -> ScalarE activation, half 1 -> VectorE int16 Schraudolph
(bf16_bits = int16(score*A + B)). Normalization per 512-token PV chunk:
reciprocal_approx_fast of the PSUM den row + gpsimd partition_broadcast +
multiply, sent straight to the AllToAll input (head-split -> token-split;
every A2A byte useful, no blending). A2A#1 (heads 0,1) fires mid-attention;
A2A#2 (heads 2,3) right after the last chunk; o_proj GEMM for the first
head-half hides A2A#2. wo is streamed from DRAM (no SBUF residency).

Self-contained: hardcodes all shapes; only imports the concourse toolchain.
"""

import sys

for _p in ("/opt/trn_rl_repo", "/root/.axon_site/_ro/trn_rl_repo"):
    if _p not in sys.path:
        sys.path.insert(0, _p)

import math

import numpy as np
import ml_dtypes

import concourse.bass as bass
import concourse.mybir as mybir
import concourse.tile as tile
from concourse import bacc
from concourse.bass_utils import run_bass_kernel_spmd
from concourse.masks import make_identity

B, S, HID = 2, 2048, 2048
NH, NKV, HD = 32, 8, 64
GROUPS = NH // NKV
ROPE_BASE = 10000.0
NCORES = 8

BF = mybir.dt.bfloat16
F32 = mybir.dt.float32
I16 = mybir.dt.int16

NB = S // 128  # 16 k strip blocks
LQ = [S - 128 * j for j in range(NB)]
OFFX = [0]
for _j in range(NB):
    OFFX.append(OFFX[-1] + LQ[_j])
TOT = OFFX[-1]  # 17408

LOG2E = 1.4426950408889634
SCALE = 1.0 / math.sqrt(HD)
SCH_A = SCALE * LOG2E * 128.0
SCH_B = 127.0 * 128.0 - 5.6

_CACHED = {}


def _build_nc():
    nc = bacc.Bacc("TRN2", target_bir_lowering=False, debug=False,
                   num_devices=NCORES)

    hsT = nc.declare_dram_parameter("hsT", [2, 17, 128, S], BF, isOutput=False)
    wq = nc.declare_dram_parameter("wq", [17, 128, 256], BF, isOutput=False)
    wkv0 = nc.declare_dram_parameter("wkv0", [17, 128, 128], BF,
                                     isOutput=False)
    wkv1 = nc.declare_dram_parameter("wkv1", [17, 128, 128], BF,
                                     isOutput=False)
    wo = nc.declare_dram_parameter("wo", [16, 128, HID], BF, isOutput=False)
    cos2 = nc.declare_dram_parameter("cos2", [128, S], BF, isOutput=False)
    sin2 = nc.declare_dram_parameter("sin2", [128, S], BF, isOutput=False)
    mb = nc.declare_dram_parameter("maskbin", [128, 128], BF, isOutput=False)
    ones2d = nc.declare_dram_parameter("ones2", [2, 128], BF, isOutput=False)
    out_part = nc.declare_dram_parameter("out_part", [512, HID], BF,
                                         isOutput=True)

    MULT = mybir.AluOpType.mult
    ADD = mybir.AluOpType.add
    EXP = mybir.ActivationFunctionType.Exp

    with tile.TileContext(nc) as tc:
        with tc.tile_pool(name="pers", bufs=1) as pers, \
             tc.tile_pool(name="dram", bufs=1, space="DRAM") as dram:
            # qT2[dim-of-head-pair, pair, batch, pos]; halves = heads 2p/2p+1
            qT2 = pers.tile([128, 2, 2, S], BF)
            # kT2[dup kv dims (both halves identical), batch, pos]
            kT2 = pers.tile([128, 2, S], BF)
            # v_aug[kpos, batch, block, dim+ones]
            v_aug = pers.tile([128, 2, NB, 65], BF)
            idn = pers.tile([128, 128], BF)
            make_identity(nc, idn)
            maskb = pers.tile([128, 128], BF)
            nc.sync.dma_start(out=maskb[:], in_=mb[:])
            nc.vector.memset(v_aug[:, :, :, 64:65], 1.0)
            # block-diagonal ones for the PE denominator broadcast:
            # rb = ones2.T @ [rec_h0; rec_h1] -> rows 0:64 = rec_h0, 64:128 =
            # rec_h1
            ones2 = pers.tile([2, 128], BF)
            nc.sync.dma_start(out=ones2[:], in_=ones2d[:])
            # gathered attention rows for o_proj (one tile per A2A)
            attg0 = pers.tile([128, 8, 512], BF)
            attg1 = pers.tile([128, 8, 512], BF)

            a2aA_in = dram.tile([NCORES, 128, 512], BF)
            a2aA_out = dram.tile([NCORES, 128, 512], BF)
            a2aB_in = dram.tile([NCORES, 128, 512], BF)
            a2aB_out = dram.tile([NCORES, 128, 512], BF)

            # ---------------- qkv projection + RoPE (bf16) ----------------
            with tc.tile_pool(name="proj", bufs=1) as pj, \
                 tc.tile_pool(name="projp", bufs=1, space="PSUM") as pjp:
                hsT_sb = pj.tile([128, 2, 17, S], BF)
                cos_sb = pj.tile([128, S], BF)
                sin_sb = pj.tile([128, S], BF)
                vT2 = pj.tile([128, S], BF)  # rows: [v(b1) | v(b0)]

                # startup loads spread over the 3 DMA trigger queues; order
                # matches consumption (kv(b0) first). wkv0 fully pre-fetched
                # first on gpsimd (bufs=17: no buffer reuse -> no pre-emission
                # race) so the first matmuls aren't queued behind hsT chunks.
                wkv0_sb = []
                for ki in range(17):
                    wt = pj.tile([128, 128], BF, tag="wkv0", bufs=17,
                                 name=f"wkv0_{ki}")
                    nc.gpsimd.dma_start(out=wt[:], in_=wkv0[ki])
                    wkv0_sb.append(wt)
                for ki in range(17):
                    eng = nc.sync if ki % 2 == 0 else nc.gpsimd
                    eng.dma_start(out=hsT_sb[:, 0, ki, :], in_=hsT[0, ki])
                for ki in range(17):
                    nc.scalar.dma_start(out=hsT_sb[:, 1, ki, :],
                                        in_=hsT[1, ki])
                nc.sync.dma_start(out=cos_sb[:], in_=cos2[:])
                nc.sync.dma_start(out=sin_sb[:], in_=sin2[:])

                def rope(pq, lo, hi, dst_slices, sl):
                    """RoPE rows [lo:hi) of psum pq; write to dst slices."""
                    n = hi - lo
                    tmp = pj.tile([128, 512], BF, tag="ropetmp", bufs=2)
                    pairs = [(0, 32), (32, 0), (64, 96), (96, 64)]
                    for (a, bb) in pairs:
                        if a >= n:
                            continue
                        nc.vector.tensor_tensor(
                            out=tmp[lo + a:lo + a + 32, :],
                            in0=pq[lo + bb:lo + bb + 32, :],
                            in1=sin_sb[lo + a:lo + a + 32, sl], op=MULT)
                    tmp2 = pj.tile([128, 512], BF, tag="ropetmp2", bufs=2)
                    nc.vector.tensor_tensor(out=tmp2[lo:hi, :],
                                            in0=pq[lo:hi, :],
                                            in1=cos_sb[lo:hi, sl], op=MULT)
                    for dst in dst_slices:
                        nc.vector.tensor_tensor(out=dst, in0=tmp2[lo:hi, :],
                                                in1=tmp[lo:hi, :], op=ADD)

                # kv units: b0 uses [k|v] weights, b1 uses [v|k]
                for b in range(2):
                    accs = [pjp.tile([128, 512], F32, tag="acc", bufs=6,
                                     name=f"acckv_{b}_{Q}") for Q in range(4)]
                    for ki in range(17):
                        if b == 0:
                            wt = wkv0_sb[ki]
                        else:
                            wt = pj.tile([128, 128], BF, tag="wkv1", bufs=4,
                                         name=f"wkv1_{ki}")
                            nc.gpsimd.dma_start(out=wt[:], in_=wkv1[ki])
                        for Q in range(4):
                            nc.tensor.matmul(
                                accs[Q][:], lhsT=wt[:],
                                rhs=hsT_sb[:, b, ki, 512 * Q:512 * Q + 512],
                                start=(ki == 0), stop=(ki == 16))
                    for Q in range(4):
                        sl = slice(512 * Q, 512 * Q + 512)
                        pq = accs[Q]
                        if b == 0:
                            # rows 0:64 = k(b0) -> kT2[0:64, 0]
                            rope(pq, 0, 64, [kT2[0:64, 0, sl]], sl)
                            nc.scalar.copy(vT2[64:128, sl], pq[64:128, :])
                        else:
                            # rows 64:128 = k(b1) -> kT2[64:128, 1]
                            rope(pq, 64, 128, [kT2[64:128, 1, sl]], sl)
                            nc.scalar.copy(vT2[0:64, sl], pq[0:64, :])
                    # duplicate kv head across partition halves
                    # (partition-shifted DVE copies, 512 cols each)
                    for Q in range(4):
                        sl = slice(512 * Q, 512 * Q + 512)
                        if b == 0:
                            nc.vector.tensor_copy(kT2[64:128, 0, sl],
                                                  kT2[0:64, 0, sl])
                        else:
                            nc.vector.tensor_copy(kT2[0:64, 1, sl],
                                                  kT2[64:128, 1, sl])

                # v: vT2 rows [v(b1)|v(b0)] -> v_aug[kpos, batch, block, dim]
                for kb in range(NB):
                    pvt = pjp.tile([128, 128], BF, tag="vt", bufs=2,
                                   name=f"pvt_{kb}")
                    nc.tensor.transpose(pvt[:], vT2[:, 128 * kb:128 * kb + 128],
                                        idn[:])
                    nc.vector.tensor_copy(v_aug[:, 1, kb, 0:64], pvt[:, 0:64])
                    nc.vector.tensor_copy(v_aug[:, 0, kb, 0:64],
                                          pvt[:, 64:128])

                # q units: pair p holds heads (2p, 2p+1) in halves.
                # Per-Q ki loops so rope(Q) overlaps the next Q's matmuls
                # instead of piling up at the unit end (hsT is fully
                # resident by the time q units run).
                wq_sb = {}
                for p in range(2):
                    for b in range(2):
                        for Q in range(4):
                            acc = pjp.tile([128, 512], F32, tag="acc",
                                           bufs=6, name=f"accq_{p}_{b}_{Q}")
                            for ki in range(17):
                                if (p, ki) not in wq_sb:
                                    wt = pj.tile([128, 128], BF, tag="wq",
                                                 bufs=17,
                                                 name=f"wq_{p}_{ki}")
                                    nc.scalar.dma_start(
                                        out=wt[:],
                                        in_=wq[ki, :, 128 * p:128 * p + 128])
                                    wq_sb[(p, ki)] = wt
                                nc.tensor.matmul(
                                    acc[:], lhsT=wq_sb[(p, ki)][:],
                                    rhs=hsT_sb[:, b, ki,
                                               512 * Q:512 * Q + 512],
                                    start=(ki == 0), stop=(ki == 16))
                            sl = slice(512 * Q, 512 * Q + 512)
                            rope(acc, 0, 128, [qT2[:, p, b, sl]], sl)

            # ---------------- attention ----------------
            with tc.tile_pool(name="att", bufs=1) as at, \
                 tc.tile_pool(name="attp", bufs=1, space="PSUM") as atp:

                def qk_strip(p, b, j, probsL, probsH):
                    q0 = 128 * j
                    L = LQ[j]
                    for cb in range(0, L, 1024):
                        w = min(1024, L - cb)
                        for h, probs in ((0, probsL), (1, probsH)):
                            ps = atp.tile([128, 1024], F32, tag="sc", bufs=3,
                                          name=f"sc_{p}_{b}_{j}_{cb}_{h}")
                            for m0 in (0, 512):
                                if m0 >= w:
                                    continue
                                mw = min(512, w - m0)
                                nc.tensor.matmul(
                                    ps[:, m0:m0 + mw],
                                    lhsT=kT2[64 * h:64 * h + 64, b,
                                             q0:q0 + 128],
                                    rhs=qT2[64 * h:64 * h + 64, p, b,
                                            q0 + cb + m0:q0 + cb + m0 + mw],
                                    start=True, stop=True)
                            dst = probs[:, OFFX[j] + cb:OFFX[j] + cb + w]
                            # engine split balances Act vs DVE load
                            if h == 0 or j in (8, 13, 14, 15):
                                nc.scalar.activation(dst, ps[:, 0:w], EXP,
                                                     scale=SCALE)
                            else:
                                nc.vector.tensor_scalar(
                                    out=dst.bitcast(I16), in0=ps[:, 0:w],
                                    scalar1=SCH_A, scalar2=SCH_B,
                                    op0=MULT, op1=ADD)
                            if cb == 0:
                                # causal mask on the diagonal block
                                nc.vector.tensor_tensor(
                                    out=probs[:, OFFX[j]:OFFX[j] + 128],
                                    in0=probs[:, OFFX[j]:OFFX[j] + 128],
                                    in1=maskb[:], op=MULT)

                def pv_chunk(p, b, h, c, probs, den):
                    """PV for 512-token chunk c; den row h of the pair's den
                    tile is filled; pvt copied to SBUF aa (releases PSUM)."""
                    pvt = atp.tile([65, 512], F32, tag="pv", bufs=2,
                                   name=f"pv_{p}_{b}_{h}_{c}")
                    for j in range(4 * c + 4):
                        if j <= 4 * c:
                            col = OFFX[j] + 512 * c - 128 * j
                            nc.tensor.matmul(
                                pvt[:, 0:512], lhsT=v_aug[:, b, j, :],
                                rhs=probs[:, col:col + 512],
                                start=(j == 0), stop=(j == 4 * c + 3))
                        else:
                            d0 = 128 * (j - 4 * c)
                            nc.tensor.matmul(
                                pvt[:, d0:512], lhsT=v_aug[:, b, j, :],
                                rhs=probs[:, OFFX[j]:OFFX[j] + 512 - d0],
                                start=False, stop=(j == 4 * c + 3))
                    aa = at.tile([64, 512], BF, tag="aa", bufs=4,
                                 name=f"aa_{p}_{b}_{h}_{c}")
                    nc.scalar.copy(aa[:], pvt[0:64, :])
                    nc.scalar.copy(den[0:1, h, :], pvt[64:65, :])
                    return aa

                def norm_send(p, b, c, den, aa0, aa1):
                    """Reciprocal + rank-1 PE broadcast of the denominators,
                    normalize, send to dest core 4b+c."""
                    rec = at.tile([1, 2, 512], F32, tag="rec", bufs=1,
                                  name=f"rec_{p}_{b}_{c}")
                    nc.vector.reciprocal_approx_fast(out=rec[:], in_=den[:])
                    rec_b = at.tile([1, 2, 512], BF, tag="recb", bufs=1,
                                    name=f"recb_{p}_{b}_{c}")
                    nc.scalar.copy(rec_b[:], rec[:])
                    a2a_in = a2aA_in if p == 0 else a2aB_in
                    for h, aa in ((0, aa0), (1, aa1)):
                        rb = atp.tile([64, 512], F32, tag="sc", bufs=3,
                                      name=f"rb_{p}_{b}_{h}_{c}")
                        nc.tensor.matmul(rb[:], lhsT=ones2[0:1, 0:64],
                                         rhs=rec_b[0:1, h, :],
                                         start=True, stop=True)
                        an = at.tile([64, 512], BF, tag="an", bufs=2,
                                     name=f"an_{p}_{b}_{h}_{c}")
                        nc.vector.tensor_tensor(
                            out=an[:], in0=aa[:], in1=rb[:], op=MULT)
                        nc.gpsimd.dma_start(
                            out=a2a_in[4 * b + c, 64 * h:64 * h + 64, :],
                            in_=an[:])

                pending = None
                for p in range(2):
                    for b in range(2):
                        probsL = at.tile([128, TOT], BF, tag="probsL", bufs=2,
                                         name=f"probsL_{p}_{b}")
                        probsH = at.tile([128, TOT], BF, tag="probsH", bufs=2,
                                         name=f"probsH_{p}_{b}")
                        # all QK strips first, then all PV chunks: PV never
                        # reads probs regions while exp is writing them, and
                        # norm_send is deferred one chunk so the rb matmul
                        # never stalls the in-order PE queue on recip
                        for j in range(NB):
                            qk_strip(p, b, j, probsL, probsH)
                            if j == 1 and pending is not None:
                                norm_send(*pending)
                                pending = None
                                if (p, b) == (1, 0):
                                    # pair 0 fully sent -> first AllToAll
                                    nc.gpsimd.collective_compute(
                                        "AllToAll", mybir.AluOpType.bypass,
                                        replica_groups=[list(range(NCORES))],
                                        ins=[a2aA_in.opt()],
                                        outs=[a2aA_out.opt()])
                                    nc.sync.dma_start(
                                        out=attg0[:],
                                        in_=a2aA_out.rearrange(
                                            "s p n -> p s n"))
                        for c in range(4):
                            den = at.tile([1, 2, 512], F32, tag="den",
                                          bufs=2, name=f"den_{p}_{b}_{c}")
                            aa0 = pv_chunk(p, b, 0, c, probsL, den)
                            aa1 = pv_chunk(p, b, 1, c, probsH, den)
                            if pending is not None:
                                norm_send(*pending)
                            pending = (p, b, c, den, aa0, aa1)
                norm_send(*pending)
                nc.gpsimd.collective_compute(
                    "AllToAll", mybir.AluOpType.bypass,
                    replica_groups=[list(range(NCORES))],
                    ins=[a2aB_in.opt()], outs=[a2aB_out.opt()])
                # readback on the (idle) sync queue so the A2A wait
                # doesn't block exp/copy work on other queues
                nc.sync.dma_start(
                    out=attg1[:],
                    in_=a2aB_out.rearrange("s p n -> p s n"))

            # ---------------- o_proj (my 512 tokens, all 2048 od) ---------
            # passes: (hf, od-half); wo streamed from DRAM; hf=0 GEMMs hide
            # A2A#2.
            with tc.tile_pool(name="op", bufs=1) as po, \
                 tc.tile_pool(name="opp", bufs=1, space="PSUM") as pop:
                part = {}
                for hf, attg in ((0, attg0), (1, attg1)):
                    for od in range(2):
                        ods = slice(1024 * od, 1024 * od + 1024)
                        psos = [pop.tile([128, 1024], F32, tag="po", bufs=4,
                                         name=f"pso_{hf}_{od}_{st}")
                                for st in range(4)]
                        for k8 in range(8):
                            wot = po.tile([128, 1024], BF, tag="wo", bufs=4,
                                          name=f"wo_{hf}_{od}_{k8}")
                            nc.scalar.dma_start(out=wot[:],
                                                in_=wo[8 * hf + k8, :, ods])
                            for st in range(4):
                                lhsT = attg[:, k8, 128 * st:128 * st + 128]
                                for u in range(2):
                                    nc.tensor.matmul(
                                        psos[st][:, 512 * u:512 * u + 512],
                                        lhsT=lhsT,
                                        rhs=wot[:, 512 * u:512 * u + 512],
                                        start=(k8 == 0), stop=(k8 == 7))
                        for st in range(4):
                            if hf == 0:
                                pt = po.tile([128, 1024], BF, tag="part",
                                             bufs=8, name=f"part_{od}_{st}")
                                nc.scalar.copy(pt[:], psos[st][:])
                                part[(od, st)] = pt
                            else:
                                oso = po.tile([128, 1024], BF, tag="oso",
                                              bufs=2, name=f"oso_{od}_{st}")
                                nc.vector.tensor_tensor(
                                    out=oso[:], in0=part[(od, st)][:],
                                    in1=psos[st][:], op=ADD)
                                nc.gpsimd.dma_start(
                                    out=out_part[128 * st:128 * st + 128,
                                                 ods],
                                    in_=oso[:])

    nc.compile()
    return nc


def _rope_tables():
    inv_freq = 1.0 / (ROPE_BASE ** (np.arange(0, HD, 2, dtype=np.float32) / HD))
    t = np.arange(S, dtype=np.float32)
    freqs = np.outer(t, inv_freq).astype(np.float32)  # [S, 32]
    cosT = np.cos(freqs).T  # [32, S]
    sinT = np.sin(freqs).T
    cos64 = np.concatenate([cosT, cosT], axis=0)          # [64, S]
    sin64 = np.concatenate([-sinT, sinT], axis=0)         # signed
    bf = ml_dtypes.bfloat16
    cos2 = np.concatenate([cos64, cos64], axis=0).astype(bf)
    sin2 = np.concatenate([sin64, sin64], axis=0).astype(bf)
    return cos2, sin2


def _np_reference(hidden_states, attention_mask, q_w, q_b, k_w, k_b, v_w, v_b,
                  o_w):
    hs = hidden_states.astype(np.float64)
    q = hs @ q_w.T.astype(np.float64) + q_b
    k = hs @ k_w.T.astype(np.float64) + k_b
    v = hs @ v_w.T.astype(np.float64) + v_b
    q = q.reshape(B, S, NH, HD).transpose(0, 2, 1, 3)
    k = k.reshape(B, S, NKV, HD).transpose(0, 2, 1, 3)
    v = v.reshape(B, S, NKV, HD).transpose(0, 2, 1, 3)
    inv_freq = 1.0 / (ROPE_BASE ** (np.arange(0, HD, 2) / HD))
    t = np.arange(S)
    freqs = np.outer(t, inv_freq)
    emb = np.concatenate([freqs, freqs], axis=-1)
    cos, sin = np.cos(emb), np.sin(emb)

    def rot(x):
        h = x.shape[-1] // 2
        return np.concatenate([-x[..., h:], x[..., :h]], axis=-1)

    q = q * cos + rot(q) * sin
    k = k * cos + rot(k) * sin
    k = np.repeat(k, GROUPS, axis=1)
    v = np.repeat(v, GROUPS, axis=1)
    sc = np.einsum("bhqd,bhkd->bhqk", q, k) / math.sqrt(HD)
    sc = sc + attention_mask.astype(np.float64)
    sc = sc - sc.max(axis=-1, keepdims=True)
    p = np.exp(sc)
    p = p / p.sum(axis=-1, keepdims=True)
    out = np.einsum("bhqk,bhkd->bhqd", p, v)
    out = out.transpose(0, 2, 1, 3).reshape(B, S, HID)
    return (out @ o_w.T.astype(np.float64)).astype(np.float32)


def _pack_chunks17(mat, bias):
    """[2048, M] weights + [M] bias -> [17, 128, M] with bias in row 0 of
    chunk 16."""
    m = mat.shape[1]
    out = np.zeros((17, 128, m), dtype=mat.dtype)
    out[:16] = mat.reshape(16, 128, m)
    out[16, 0, :] = bias
    return out


def _make_in_maps(inputs):
    hs = np.asarray(inputs["hidden_states"], np.float32)
    q_w = np.asarray(inputs["q_w"], np.float32)
    q_b = np.asarray(inputs["q_b"], np.float32)
    k_w = np.asarray(inputs["k_w"], np.float32)
    k_b = np.asarray(inputs["k_b"], np.float32)
    v_w = np.asarray(inputs["v_w"], np.float32)
    v_b = np.asarray(inputs["v_b"], np.float32)
    o_w = np.asarray(inputs["o_w"], np.float32)
    mask = np.asarray(inputs["attention_mask"], np.float32)
    m2 = mask[0, 0]

    bf = ml_dtypes.bfloat16
    cos2, sin2 = _rope_tables()
    # binary mask for the diagonal block, transposed orientation [k, q]
    maskbin = (m2[0:128, 0:128].T == 0.0).astype(bf)

    # wo rows in A2A arrival order: chunk 8*hf + s = heads (4s+2hf, 4s+2hf+1)
    o_wT = np.ascontiguousarray(o_w.T.astype(bf))  # [2048 in, 2048 out]
    rows = []
    for hf in range(2):
        for s in range(8):
            h = 4 * s + 2 * hf
            rows.extend(range(64 * h, 64 * h + 64))
            rows.extend(range(64 * (h + 1), 64 * (h + 1) + 64))
    wo_np = o_wT[np.array(rows)].reshape(16, 128, HID)

    hsT_packed = np.zeros((2, 17, 128, S), dtype=bf)
    for b in range(B):
        hsT_packed[b, :16] = np.ascontiguousarray(hs[b].T).astype(bf).reshape(
            16, 128, S)
        hsT_packed[b, 16, 0, :] = 1.0

    q_wT = np.ascontiguousarray(q_w.T).astype(bf)  # [2048, 2048]
    k_wT = np.ascontiguousarray(k_w.T).astype(bf)  # [2048, 512]
    v_wT = np.ascontiguousarray(v_w.T).astype(bf)

    # block-diagonal broadcast helper: rb = ones2.T @ [rec0; rec1]
    ones2_np = np.zeros((2, 128), dtype=bf)
    ones2_np[0, 0:64] = 1.0
    ones2_np[1, 64:128] = 1.0

    in_maps = []
    for c in range(NCORES):
        kv = slice(64 * c, 64 * c + 64)
        kvw0 = np.concatenate([k_wT[:, kv], v_wT[:, kv]], axis=1)
        kvw1 = np.concatenate([v_wT[:, kv], k_wT[:, kv]], axis=1)
        kvb0 = np.concatenate([k_b[kv], v_b[kv]]).astype(bf)
        kvb1 = np.concatenate([v_b[kv], k_b[kv]]).astype(bf)
        in_maps.append({
            "hsT": hsT_packed,
            "wq": _pack_chunks17(q_wT[:, 256 * c:256 * c + 256],
                                 q_b[256 * c:256 * c + 256].astype(bf)),
            "wkv0": _pack_chunks17(kvw0, kvb0),
            "wkv1": _pack_chunks17(kvw1, kvb1),
            "wo": wo_np,
            "cos2": cos2,
            "sin2": sin2,
            "maskbin": maskbin,
            "ones2": ones2_np,
        })
    return in_maps


def kernel(**inputs):
    mask = np.asarray(inputs["attention_mask"], np.float32)
    m2 = mask[0, 0]
    causal_ok = bool(
        np.all(m2[np.tril_indices(S)] == 0.0)
        and np.all(m2[np.triu_indices(S, 1)] < -1e8))
    if not causal_ok:
        return _np_reference(
            np.asarray(inputs["hidden_states"], np.float32), mask,
            *(np.asarray(inputs[k], np.float32)
              for k in ("q_w", "q_b", "k_w", "k_b", "v_w", "v_b", "o_w")))

    if "nc" not in _CACHED:
        _CACHED["nc"] = _build_nc()
    nc = _CACHED["nc"]
    in_maps = _make_in_maps(inputs)

    res = run_bass_kernel_spmd(nc, in_maps, list(range(NCORES)))

    out = np.empty((B, S, HID), dtype=np.float32)
    for c in range(NCORES):
        b, g = c // 4, c % 4
        out[b, 512 * g:512 * g + 512, :] = np.asarray(
            res.results[c]["out_part"], np.float32)
    return out


# revision 35
# speedup vs baseline: 1.1302x; 1.0063x over previous
"""GroupedQueryAttention Trainium2 kernel (8 NeuronCores).

Sharding: core c -> q-heads [4c,4c+4), kv-head c, BOTH batches (tensor
parallel 8-way on heads). Output tokens: batch c//4, slice 512*(c%4).

Per core: qkv projection (bias via ones-row 17th contraction chunk) + RoPE;
flash-style causal attention in transposed-score orientation probsT[k,q],
head-PAIRED on the PE: pair p in {0,1} holds heads (2p, 2p+1) in partition
halves, batch b processed as separate super-unit (p,b). The kv head is
shared by both halves, so kT2 is duplicated across partition halves via an
SBUF->SBUF DMA. Denominators via a ones-column appended to V. Exp split:
half 0 -> ScalarE activation, half 1 -> VectorE int16 Schraudolph
(bf16_bits = int16(score*A + B)). Normalization per 512-token PV chunk:
reciprocal_approx_fast of the PSUM den row + gpsimd partition_broadcast +
multiply, sent straight to the AllToAll input (head-split -> token-split;
every A2A byte useful, no blending). A2A#1 (heads 0,1) fires mid-attention;
A2A#2 (heads 2,3) right after the last chunk; o_proj GEMM for the first
head-half hides A2A#2. wo is streamed from DRAM (no SBUF residency).

Self-contained: hardcodes all shapes; only imports the concourse toolchain.
"""

import sys

for _p in ("/opt/trn_rl_repo", "/root/.axon_site/_ro/trn_rl_repo"):
    if _p not in sys.path:
        sys.path.insert(0, _p)

import math

import numpy as np
import ml_dtypes

import concourse.bass as bass
import concourse.mybir as mybir
import concourse.tile as tile
from concourse import bacc
from concourse.bass_utils import run_bass_kernel_spmd
from concourse.masks import make_identity

B, S, HID = 2, 2048, 2048
NH, NKV, HD = 32, 8, 64
GROUPS = NH // NKV
ROPE_BASE = 10000.0
NCORES = 8

BF = mybir.dt.bfloat16
F32 = mybir.dt.float32
I16 = mybir.dt.int16

NB = S // 128  # 16 k strip blocks
LQ = [S - 128 * j for j in range(NB)]
OFFX = [0]
for _j in range(NB):
    OFFX.append(OFFX[-1] + LQ[_j])
TOT = OFFX[-1]  # 17408

LOG2E = 1.4426950408889634
SCALE = 1.0 / math.sqrt(HD)
SCH_A = SCALE * LOG2E * 128.0
SCH_B = 127.0 * 128.0 - 5.6

_CACHED = {}


def _build_nc():
    nc = bacc.Bacc("TRN2", target_bir_lowering=False, debug=False,
                   num_devices=NCORES)

    hsT = nc.declare_dram_parameter("hsT", [2, 17, 128, S], BF, isOutput=False)
    wq = nc.declare_dram_parameter("wq", [17, 128, 256], BF, isOutput=False)
    wkv0 = nc.declare_dram_parameter("wkv0", [17, 128, 128], BF,
                                     isOutput=False)
    wkv1 = nc.declare_dram_parameter("wkv1", [17, 128, 128], BF,
                                     isOutput=False)
    wo = nc.declare_dram_parameter("wo", [16, 128, HID], BF, isOutput=False)
    cos2 = nc.declare_dram_parameter("cos2", [128, S], BF, isOutput=False)
    sin2 = nc.declare_dram_parameter("sin2", [128, S], BF, isOutput=False)
    mb = nc.declare_dram_parameter("maskbin", [128, 128], BF, isOutput=False)
    ones2d = nc.declare_dram_parameter("ones2", [2, 128], BF, isOutput=False)
    out_part = nc.declare_dram_parameter("out_part", [512, HID], BF,
                                         isOutput=True)

    MULT = mybir.AluOpType.mult
    ADD = mybir.AluOpType.add
    EXP = mybir.ActivationFunctionType.Exp

    with tile.TileContext(nc) as tc:
        with tc.tile_pool(name="pers", bufs=1) as pers, \
             tc.tile_pool(name="dram", bufs=1, space="DRAM") as dram:
            # qT2[dim-of-head-pair, pair, batch, pos]; halves = heads 2p/2p+1
            qT2 = pers.tile([128, 2, 2, S], BF)
            # kT2[dup kv dims (both halves identical), batch, pos]
            kT2 = pers.tile([128, 2, S], BF)
            # v_aug[kpos, batch, block, dim+ones]
            v_aug = pers.tile([128, 2, NB, 65], BF)
            idn = pers.tile([128, 128], BF)
            make_identity(nc, idn)
            maskb = pers.tile([128, 128], BF)
            nc.sync.dma_start(out=maskb[:], in_=mb[:])
            nc.vector.memset(v_aug[:, :, :, 64:65], 1.0)
            # block-diagonal ones for the PE denominator broadcast:
            # rb = ones2.T @ [rec_h0; rec_h1] -> rows 0:64 = rec_h0, 64:128 =
            # rec_h1
            ones2 = pers.tile([2, 128], BF)
            nc.sync.dma_start(out=ones2[:], in_=ones2d[:])
            # gathered attention rows for o_proj (one tile per A2A)
            attg0 = pers.tile([128, 8, 512], BF)
            attg1 = pers.tile([128, 8, 512], BF)

            a2aA_in = dram.tile([NCORES, 128, 512], BF)
            a2aA_out = dram.tile([NCORES, 128, 512], BF)
            a2aB_in = dram.tile([NCORES, 128, 512], BF)
            a2aB_out = dram.tile([NCORES, 128, 512], BF)

            # ---------------- qkv projection + RoPE (bf16) ----------------
            with tc.tile_pool(name="proj", bufs=1) as pj, \
                 tc.tile_pool(name="projp", bufs=1, space="PSUM") as pjp:
                hsT_sb = pj.tile([128, 2, 17, S], BF)
                cos_sb = pj.tile([128, S], BF)
                sin_sb = pj.tile([128, S], BF)
                vT2 = pj.tile([128, S], BF)  # rows: [v(b1) | v(b0)]

                # startup loads spread over the 3 DMA trigger queues; order
                # matches consumption (kv(b0) first). wkv0 fully pre-fetched
                # first on gpsimd (bufs=17: no buffer reuse -> no pre-emission
                # race) so the first matmuls aren't queued behind hsT chunks.
                wkv0_sb = []
                for ki in range(17):
                    wt = pj.tile([128, 128], BF, tag="wkv0", bufs=17,
                                 name=f"wkv0_{ki}")
                    nc.gpsimd.dma_start(out=wt[:], in_=wkv0[ki])
                    wkv0_sb.append(wt)
                for ki in range(17):
                    eng = nc.sync if ki % 2 == 0 else nc.gpsimd
                    eng.dma_start(out=hsT_sb[:, 0, ki, :], in_=hsT[0, ki])
                for ki in range(17):
                    nc.scalar.dma_start(out=hsT_sb[:, 1, ki, :],
                                        in_=hsT[1, ki])
                nc.sync.dma_start(out=cos_sb[:], in_=cos2[:])
                nc.sync.dma_start(out=sin_sb[:], in_=sin2[:])

                def rope(pq, lo, hi, dst_slices, sl):
                    """RoPE rows [lo:hi) of psum pq; write to dst slices."""
                    n = hi - lo
                    tmp = pj.tile([128, 512], BF, tag="ropetmp", bufs=2)
                    pairs = [(0, 32), (32, 0), (64, 96), (96, 64)]
                    for (a, bb) in pairs:
                        if a >= n:
                            continue
                        nc.vector.tensor_tensor(
                            out=tmp[lo + a:lo + a + 32, :],
                            in0=pq[lo + bb:lo + bb + 32, :],
                            in1=sin_sb[lo + a:lo + a + 32, sl], op=MULT)
                    tmp2 = pj.tile([128, 512], BF, tag="ropetmp2", bufs=2)
                    nc.vector.tensor_tensor(out=tmp2[lo:hi, :],
                                            in0=pq[lo:hi, :],
                                            in1=cos_sb[lo:hi, sl], op=MULT)
                    for dst in dst_slices:
                        nc.vector.tensor_tensor(out=dst, in0=tmp2[lo:hi, :],
                                                in1=tmp[lo:hi, :], op=ADD)

                # kv units: b0 uses [k|v] weights, b1 uses [v|k]
                for b in range(2):
                    accs = [pjp.tile([128, 512], F32, tag="acc", bufs=6,
                                     name=f"acckv_{b}_{Q}") for Q in range(4)]
                    for ki in range(17):
                        if b == 0:
                            wt = wkv0_sb[ki]
                        else:
                            wt = pj.tile([128, 128], BF, tag="wkv1", bufs=4,
                                         name=f"wkv1_{ki}")
                            nc.gpsimd.dma_start(out=wt[:], in_=wkv1[ki])
                        for Q in range(4):
                            nc.tensor.matmul(
                                accs[Q][:], lhsT=wt[:],
                                rhs=hsT_sb[:, b, ki, 512 * Q:512 * Q + 512],
                                start=(ki == 0), stop=(ki == 16))
                    for Q in range(4):
                        sl = slice(512 * Q, 512 * Q + 512)
                        pq = accs[Q]
                        if b == 0:
                            # rows 0:64 = k(b0) -> kT2[0:64, 0]
                            rope(pq, 0, 64, [kT2[0:64, 0, sl]], sl)
                            nc.scalar.copy(vT2[64:128, sl], pq[64:128, :])
                        else:
                            # rows 64:128 = k(b1) -> kT2[64:128, 1]
                            rope(pq, 64, 128, [kT2[64:128, 1, sl]], sl)
                            nc.scalar.copy(vT2[0:64, sl], pq[0:64, :])
                    # duplicate kv head across partition halves
                    # (partition-shifted DVE copies, 512 cols each)
                    for Q in range(4):
                        sl = slice(512 * Q, 512 * Q + 512)
                        if b == 0:
                            nc.vector.tensor_copy(kT2[64:128, 0, sl],
                                                  kT2[0:64, 0, sl])
                        else:
                            nc.vector.tensor_copy(kT2[0:64, 1, sl],
                                                  kT2[64:128, 1, sl])

                # v: vT2 rows [v(b1)|v(b0)] -> v_aug[kpos, batch, block, dim]
                for kb in range(NB):
                    pvt = pjp.tile([128, 128], BF, tag="vt", bufs=2,
                                   name=f"pvt_{kb}")
                    nc.tensor.transpose(pvt[:], vT2[:, 128 * kb:128 * kb + 128],
                                        idn[:])
                    nc.vector.tensor_copy(v_aug[:, 1, kb, 0:64], pvt[:, 0:64])
                    nc.vector.tensor_copy(v_aug[:, 0, kb, 0:64],
                                          pvt[:, 64:128])

                # q units: pair p holds heads (2p, 2p+1) in halves.
                # Per-Q ki loops so rope(Q) overlaps the next Q's matmuls
                # instead of piling up at the unit end (hsT is fully
                # resident by the time q units run).
                wq_sb = {}
                for p in range(2):
                    for b in range(2):
                        for Q in range(4):
                            acc = pjp.tile([128, 512], F32, tag="acc",
                                           bufs=6, name=f"accq_{p}_{b}_{Q}")
                            for ki in range(17):
                                if (p, ki) not in wq_sb:
                                    wt = pj.tile([128, 128], BF, tag="wq",
                                                 bufs=17,
                                                 name=f"wq_{p}_{ki}")
                                    nc.scalar.dma_start(
                                        out=wt[:],
                                        in_=wq[ki, :, 128 * p:128 * p + 128])
                                    wq_sb[(p, ki)] = wt
                                nc.tensor.matmul(
                                    acc[:], lhsT=wq_sb[(p, ki)][:],
                                    rhs=hsT_sb[:, b, ki,
                                               512 * Q:512 * Q + 512],
                                    start=(ki == 0), stop=(ki == 16))
                            sl = slice(512 * Q, 512 * Q + 512)
                            rope(acc, 0, 128, [qT2[:, p, b, sl]], sl)

            # ---------------- attention ----------------
            with tc.tile_pool(name="att", bufs=1) as at, \
                 tc.tile_pool(name="attp", bufs=1, space="PSUM") as atp:

                def qk_strip(p, b, j, probsL, probsH):
                    q0 = 128 * j
                    L = LQ[j]
                    for cb in range(0, L, 1024):
                        w = min(1024, L - cb)
                        # alternate PE row groups (h0 at rows 0:64, h1 at
                        # 64:128) so consecutive matmuls co-execute
                        pss = [atp.tile([128, 1024], F32, tag="sc", bufs=3,
                                        name=f"sc_{p}_{b}_{j}_{cb}_{h}")
                               for h in (0, 1)]
                        for m0 in (0, 512):
                            if m0 >= w:
                                continue
                            mw = min(512, w - m0)
                            for h in (0, 1):
                                nc.tensor.matmul(
                                    pss[h][:, m0:m0 + mw],
                                    lhsT=kT2[64 * h:64 * h + 64, b,
                                             q0:q0 + 128],
                                    rhs=qT2[64 * h:64 * h + 64, p, b,
                                            q0 + cb + m0:q0 + cb + m0 + mw],
                                    start=True, stop=True)
                        for h, probs in ((0, probsL), (1, probsH)):
                            ps = pss[h]
                            dst = probs[:, OFFX[j] + cb:OFFX[j] + cb + w]
                            # engine split balances Act vs DVE load
                            if h == 0 or j in (8, 13, 14, 15):
                                nc.scalar.activation(dst, ps[:, 0:w], EXP,
                                                     scale=SCALE)
                            else:
                                nc.vector.tensor_scalar(
                                    out=dst.bitcast(I16), in0=ps[:, 0:w],
                                    scalar1=SCH_A, scalar2=SCH_B,
                                    op0=MULT, op1=ADD)
                            if cb == 0:
                                # causal mask on the diagonal block
                                nc.vector.tensor_tensor(
                                    out=probs[:, OFFX[j]:OFFX[j] + 128],
                                    in0=probs[:, OFFX[j]:OFFX[j] + 128],
                                    in1=maskb[:], op=MULT)

                def pv_chunk(p, b, h, c, probs, den):
                    """PV for 512-token chunk c; den row h of the pair's den
                    tile is filled; pvt copied to SBUF aa (releases PSUM)."""
                    pvt = atp.tile([65, 512], F32, tag="pv", bufs=2,
                                   name=f"pv_{p}_{b}_{h}_{c}")
                    for j in range(4 * c + 4):
                        if j <= 4 * c:
                            col = OFFX[j] + 512 * c - 128 * j
                            nc.tensor.matmul(
                                pvt[:, 0:512], lhsT=v_aug[:, b, j, :],
                                rhs=probs[:, col:col + 512],
                                start=(j == 0), stop=(j == 4 * c + 3))
                        else:
                            d0 = 128 * (j - 4 * c)
                            nc.tensor.matmul(
                                pvt[:, d0:512], lhsT=v_aug[:, b, j, :],
                                rhs=probs[:, OFFX[j]:OFFX[j] + 512 - d0],
                                start=False, stop=(j == 4 * c + 3))
                    aa = at.tile([64, 512], BF, tag="aa", bufs=4,
                                 name=f"aa_{p}_{b}_{h}_{c}")
                    nc.scalar.copy(aa[:], pvt[0:64, :])
                    nc.scalar.copy(den[0:1, h, :], pvt[64:65, :])
                    return aa

                def norm_send(p, b, c, den, aa0, aa1):
                    """Reciprocal + rank-1 PE broadcast of the denominators,
                    normalize, send to dest core 4b+c."""
                    rec = at.tile([1, 2, 512], F32, tag="rec", bufs=1,
                                  name=f"rec_{p}_{b}_{c}")
                    nc.vector.reciprocal_approx_fast(out=rec[:], in_=den[:])
                    rec_b = at.tile([1, 2, 512], BF, tag="recb", bufs=1,
                                    name=f"recb_{p}_{b}_{c}")
                    nc.scalar.copy(rec_b[:], rec[:])
                    a2a_in = a2aA_in if p == 0 else a2aB_in
                    for h, aa in ((0, aa0), (1, aa1)):
                        rb = atp.tile([64, 512], F32, tag="sc", bufs=3,
                                      name=f"rb_{p}_{b}_{h}_{c}")
                        nc.tensor.matmul(rb[:], lhsT=ones2[0:1, 0:64],
                                         rhs=rec_b[0:1, h, :],
                                         start=True, stop=True)
                        an = at.tile([64, 512], BF, tag="an", bufs=2,
                                     name=f"an_{p}_{b}_{h}_{c}")
                        nc.vector.tensor_tensor(
                            out=an[:], in0=aa[:], in1=rb[:], op=MULT)
                        nc.gpsimd.dma_start(
                            out=a2a_in[4 * b + c, 64 * h:64 * h + 64, :],
                            in_=an[:])

                pending = None
                for p in range(2):
                    for b in range(2):
                        probsL = at.tile([128, TOT], BF, tag="probsL", bufs=2,
                                         name=f"probsL_{p}_{b}")
                        probsH = at.tile([128, TOT], BF, tag="probsH", bufs=2,
                                         name=f"probsH_{p}_{b}")
                        # all QK strips first, then all PV chunks: PV never
                        # reads probs regions while exp is writing them, and
                        # norm_send is deferred one chunk so the rb matmul
                        # never stalls the in-order PE queue on recip
                        for j in range(NB):
                            qk_strip(p, b, j, probsL, probsH)
                            if j == 1 and pending is not None:
                                norm_send(*pending)
                                pending = None
                                if (p, b) == (1, 0):
                                    # pair 0 fully sent -> first AllToAll
                                    nc.gpsimd.collective_compute(
                                        "AllToAll", mybir.AluOpType.bypass,
                                        replica_groups=[list(range(NCORES))],
                                        ins=[a2aA_in.opt()],
                                        outs=[a2aA_out.opt()])
                                    nc.sync.dma_start(
                                        out=attg0[:],
                                        in_=a2aA_out.rearrange(
                                            "s p n -> p s n"))
                        for c in range(4):
                            den = at.tile([1, 2, 512], F32, tag="den",
                                          bufs=2, name=f"den_{p}_{b}_{c}")
                            aa0 = pv_chunk(p, b, 0, c, probsL, den)
                            aa1 = pv_chunk(p, b, 1, c, probsH, den)
                            if pending is not None:
                                norm_send(*pending)
                            pending = (p, b, c, den, aa0, aa1)
                norm_send(*pending)
                nc.gpsimd.collective_compute(
                    "AllToAll", mybir.AluOpType.bypass,
                    replica_groups=[list(range(NCORES))],
                    ins=[a2aB_in.opt()], outs=[a2aB_out.opt()])
                # readback on the (idle) sync queue so the A2A wait
                # doesn't block exp/copy work on other queues
                nc.sync.dma_start(
                    out=attg1[:],
                    in_=a2aB_out.rearrange("s p n -> p s n"))

            # ---------------- o_proj (my 512 tokens, all 2048 od) ---------
            # passes: (hf, od-half); wo streamed from DRAM; hf=0 GEMMs hide
            # A2A#2.
            with tc.tile_pool(name="op", bufs=1) as po, \
                 tc.tile_pool(name="opp", bufs=1, space="PSUM") as pop:
                part = {}
                for hf, attg in ((0, attg0), (1, attg1)):
                    for od in range(2):
                        ods = slice(1024 * od, 1024 * od + 1024)
                        psos = [pop.tile([128, 1024], F32, tag="po", bufs=4,
                                         name=f"pso_{hf}_{od}_{st}")
                                for st in range(4)]
                        for k8 in range(8):
                            wot = po.tile([128, 1024], BF, tag="wo", bufs=4,
                                          name=f"wo_{hf}_{od}_{k8}")
                            nc.scalar.dma_start(out=wot[:],
                                                in_=wo[8 * hf + k8, :, ods])
                            for st in range(4):
                                lhsT = attg[:, k8, 128 * st:128 * st + 128]
                                for u in range(2):
                                    nc.tensor.matmul(
                                        psos[st][:, 512 * u:512 * u + 512],
                                        lhsT=lhsT,
                                        rhs=wot[:, 512 * u:512 * u + 512],
                                        start=(k8 == 0), stop=(k8 == 7))
                        for st in range(4):
                            if hf == 0:
                                pt = po.tile([128, 1024], BF, tag="part",
                                             bufs=8, name=f"part_{od}_{st}")
                                nc.scalar.copy(pt[:], psos[st][:])
                                part[(od, st)] = pt
                            else:
                                oso = po.tile([128, 1024], BF, tag="oso",
                                              bufs=2, name=f"oso_{od}_{st}")
                                nc.vector.tensor_tensor(
                                    out=oso[:], in0=part[(od, st)][:],
                                    in1=psos[st][:], op=ADD)
                                nc.gpsimd.dma_start(
                                    out=out_part[128 * st:128 * st + 128,
                                                 ods],
                                    in_=oso[:])

    nc.compile()
    return nc


def _rope_tables():
    inv_freq = 1.0 / (ROPE_BASE ** (np.arange(0, HD, 2, dtype=np.float32) / HD))
    t = np.arange(S, dtype=np.float32)
    freqs = np.outer(t, inv_freq).astype(np.float32)  # [S, 32]
    cosT = np.cos(freqs).T  # [32, S]
    sinT = np.sin(freqs).T
    cos64 = np.concatenate([cosT, cosT], axis=0)          # [64, S]
    sin64 = np.concatenate([-sinT, sinT], axis=0)         # signed
    bf = ml_dtypes.bfloat16
    cos2 = np.concatenate([cos64, cos64], axis=0).astype(bf)
    sin2 = np.concatenate([sin64, sin64], axis=0).astype(bf)
    return cos2, sin2


def _np_reference(hidden_states, attention_mask, q_w, q_b, k_w, k_b, v_w, v_b,
                  o_w):
    hs = hidden_states.astype(np.float64)
    q = hs @ q_w.T.astype(np.float64) + q_b
    k = hs @ k_w.T.astype(np.float64) + k_b
    v = hs @ v_w.T.astype(np.float64) + v_b
    q = q.reshape(B, S, NH, HD).transpose(0, 2, 1, 3)
    k = k.reshape(B, S, NKV, HD).transpose(0, 2, 1, 3)
    v = v.reshape(B, S, NKV, HD).transpose(0, 2, 1, 3)
    inv_freq = 1.0 / (ROPE_BASE ** (np.arange(0, HD, 2) / HD))
    t = np.arange(S)
    freqs = np.outer(t, inv_freq)
    emb = np.concatenate([freqs, freqs], axis=-1)
    cos, sin = np.cos(emb), np.sin(emb)

    def rot(x):
        h = x.shape[-1] // 2
        return np.concatenate([-x[..., h:], x[..., :h]], axis=-1)

    q = q * cos + rot(q) * sin
    k = k * cos + rot(k) * sin
    k = np.repeat(k, GROUPS, axis=1)
    v = np.repeat(v, GROUPS, axis=1)
    sc = np.einsum("bhqd,bhkd->bhqk", q, k) / math.sqrt(HD)
    sc = sc + attention_mask.astype(np.float64)
    sc = sc - sc.max(axis=-1, keepdims=True)
    p = np.exp(sc)
    p = p / p.sum(axis=-1, keepdims=True)
    out = np.einsum("bhqk,bhkd->bhqd", p, v)
    out = out.transpose(0, 2, 1, 3).reshape(B, S, HID)
    return (out @ o_w.T.astype(np.float64)).astype(np.float32)


def _pack_chunks17(mat, bias):
    """[2048, M] weights + [M] bias -> [17, 128, M] with bias in row 0 of
    chunk 16."""
    m = mat.shape[1]
    out = np.zeros((17, 128, m), dtype=mat.dtype)
    out[:16] = mat.reshape(16, 128, m)
    out[16, 0, :] = bias
    return out


def _make_in_maps(inputs):
    hs = np.asarray(inputs["hidden_states"], np.float32)
    q_w = np.asarray(inputs["q_w"], np.float32)
    q_b = np.asarray(inputs["q_b"], np.float32)
    k_w = np.asarray(inputs["k_w"], np.float32)
    k_b = np.asarray(inputs["k_b"], np.float32)
    v_w = np.asarray(inputs["v_w"], np.float32)
    v_b = np.asarray(inputs["v_b"], np.float32)
    o_w = np.asarray(inputs["o_w"], np.float32)
    mask = np.asarray(inputs["attention_mask"], np.float32)
    m2 = mask[0, 0]

    bf = ml_dtypes.bfloat16
    cos2, sin2 = _rope_tables()
    # binary mask for the diagonal block, transposed orientation [k, q]
    maskbin = (m2[0:128, 0:128].T == 0.0).astype(bf)

    # wo rows in A2A arrival order: chunk 8*hf + s = heads (4s+2hf, 4s+2hf+1)
    o_wT = np.ascontiguousarray(o_w.T.astype(bf))  # [2048 in, 2048 out]
    rows = []
    for hf in range(2):
        for s in range(8):
            h = 4 * s + 2 * hf
            rows.extend(range(64 * h, 64 * h + 64))
            rows.extend(range(64 * (h + 1), 64 * (h + 1) + 64))
    wo_np = o_wT[np.array(rows)].reshape(16, 128, HID)

    hsT_packed = np.zeros((2, 17, 128, S), dtype=bf)
    for b in range(B):
        hsT_packed[b, :16] = np.ascontiguousarray(hs[b].T).astype(bf).reshape(
            16, 128, S)
        hsT_packed[b, 16, 0, :] = 1.0

    q_wT = np.ascontiguousarray(q_w.T).astype(bf)  # [2048, 2048]
    k_wT = np.ascontiguousarray(k_w.T).astype(bf)  # [2048, 512]
    v_wT = np.ascontiguousarray(v_w.T).astype(bf)

    # block-diagonal broadcast helper: rb = ones2.T @ [rec0; rec1]
    ones2_np = np.zeros((2, 128), dtype=bf)
    ones2_np[0, 0:64] = 1.0
    ones2_np[1, 64:128] = 1.0

    in_maps = []
    for c in range(NCORES):
        kv = slice(64 * c, 64 * c + 64)
        kvw0 = np.concatenate([k_wT[:, kv], v_wT[:, kv]], axis=1)
        kvw1 = np.concatenate([v_wT[:, kv], k_wT[:, kv]], axis=1)
        kvb0 = np.concatenate([k_b[kv], v_b[kv]]).astype(bf)
        kvb1 = np.concatenate([v_b[kv], k_b[kv]]).astype(bf)
        in_maps.append({
            "hsT": hsT_packed,
            "wq": _pack_chunks17(q_wT[:, 256 * c:256 * c + 256],
                                 q_b[256 * c:256 * c + 256].astype(bf)),
            "wkv0": _pack_chunks17(kvw0, kvb0),
            "wkv1": _pack_chunks17(kvw1, kvb1),
            "wo": wo_np,
            "cos2": cos2,
            "sin2": sin2,
            "maskbin": maskbin,
            "ones2": ones2_np,
        })
    return in_maps


def kernel(**inputs):
    mask = np.asarray(inputs["attention_mask"], np.float32)
    m2 = mask[0, 0]
    causal_ok = bool(
        np.all(m2[np.tril_indices(S)] == 0.0)
        and np.all(m2[np.triu_indices(S, 1)] < -1e8))
    if not causal_ok:
        return _np_reference(
            np.asarray(inputs["hidden_states"], np.float32), mask,
            *(np.asarray(inputs[k], np.float32)
              for k in ("q_w", "q_b", "k_w", "k_b", "v_w", "v_b", "o_w")))

    if "nc" not in _CACHED:
        _CACHED["nc"] = _build_nc()
    nc = _CACHED["nc"]
    in_maps = _make_in_maps(inputs)

    res = run_bass_kernel_spmd(nc, in_maps, list(range(NCORES)))

    out = np.empty((B, S, HID), dtype=np.float32)
    for c in range(NCORES):
        b, g = c // 4, c % 4
        out[b, 512 * g:512 * g + 512, :] = np.asarray(
            res.results[c]["out_part"], np.float32)
    return out


# revision 36
# speedup vs baseline: 1.1404x; 1.0091x over previous
"""GroupedQueryAttention Trainium2 kernel (8 NeuronCores).

Sharding: core c -> q-heads [4c,4c+4), kv-head c, BOTH batches (tensor
parallel 8-way on heads). Output tokens: batch c//4, slice 512*(c%4).

Per core: qkv projection (bias via ones-row 17th contraction chunk) + RoPE
(per-Q accumulation chains so rope overlaps the next chain's matmuls);
flash-style causal attention in transposed-score orientation probsT[k,q],
head-PAIRED on the PE: pair p in {0,1} holds heads (2p, 2p+1) in partition
halves (PE row groups alternate h0/h64 so QK matmuls co-execute), batch b
processed as separate super-unit (p,b). The kv head is shared by both
halves, so kT2 is duplicated across partition halves via partition-shifted
DVE copies. Per unit: all QK strips first, then all PV chunks, so PV never
reads probs regions concurrently with exp writes. Denominators via a
ones-column appended to V. Exp split by half/strip across ScalarE
activation and VectorE int16 Schraudolph (bf16_bits = int16(score*A + B)).
Normalization per 512-token PV chunk, deferred one chunk so the PE never
stalls on the recip chain: reciprocal_approx_fast of the SBUF-bounced den
row + rank-1 PE broadcast (ones x rec into PSUM) + multiply, sent straight
to the AllToAll input (head-split -> token-split; every A2A byte useful,
no blending). A2A#1 (heads 0,1) fires mid-attention; A2A#2 (heads 2,3)
right after the last chunk; o_proj GEMM for the first head-half hides
A2A#2. wo is streamed from DRAM (no SBUF residency); bf16 output.

Self-contained: hardcodes all shapes; only imports the concourse toolchain.
"""

import sys

for _p in ("/opt/trn_rl_repo", "/root/.axon_site/_ro/trn_rl_repo"):
    if _p not in sys.path:
        sys.path.insert(0, _p)

import math

import numpy as np
import ml_dtypes

import concourse.bass as bass
import concourse.mybir as mybir
import concourse.tile as tile
from concourse import bacc
from concourse.bass_utils import run_bass_kernel_spmd
from concourse.masks import make_identity

B, S, HID = 2, 2048, 2048
NH, NKV, HD = 32, 8, 64
GROUPS = NH // NKV
ROPE_BASE = 10000.0
NCORES = 8

BF = mybir.dt.bfloat16
F32 = mybir.dt.float32
I16 = mybir.dt.int16

NB = S // 128  # 16 k strip blocks
LQ = [S - 128 * j for j in range(NB)]
OFFX = [0]
for _j in range(NB):
    OFFX.append(OFFX[-1] + LQ[_j])
TOT = OFFX[-1]  # 17408

LOG2E = 1.4426950408889634
SCALE = 1.0 / math.sqrt(HD)
SCH_A = SCALE * LOG2E * 128.0
SCH_B = 127.0 * 128.0 - 5.6

_CACHED = {}


def _build_nc():
    nc = bacc.Bacc("TRN2", target_bir_lowering=False, debug=False,
                   num_devices=NCORES)

    hsT = nc.declare_dram_parameter("hsT", [2, 17, 128, S], BF, isOutput=False)
    wq = nc.declare_dram_parameter("wq", [17, 128, 256], BF, isOutput=False)
    wkv0 = nc.declare_dram_parameter("wkv0", [17, 128, 128], BF,
                                     isOutput=False)
    wkv1 = nc.declare_dram_parameter("wkv1", [17, 128, 128], BF,
                                     isOutput=False)
    wo = nc.declare_dram_parameter("wo", [16, 128, HID], BF, isOutput=False)
    cos2 = nc.declare_dram_parameter("cos2", [128, S], BF, isOutput=False)
    sin2 = nc.declare_dram_parameter("sin2", [128, S], BF, isOutput=False)
    mb = nc.declare_dram_parameter("maskbin", [128, 128], BF, isOutput=False)
    ones2d = nc.declare_dram_parameter("ones2", [2, 128], BF, isOutput=False)
    out_part = nc.declare_dram_parameter("out_part", [512, HID], BF,
                                         isOutput=True)

    MULT = mybir.AluOpType.mult
    ADD = mybir.AluOpType.add
    EXP = mybir.ActivationFunctionType.Exp

    with tile.TileContext(nc) as tc:
        with tc.tile_pool(name="pers", bufs=1) as pers, \
             tc.tile_pool(name="dram", bufs=1, space="DRAM") as dram:
            # qT2[dim-of-head-pair, pair, batch, pos]; halves = heads 2p/2p+1
            qT2 = pers.tile([128, 2, 2, S], BF)
            # kT2[dup kv dims (both halves identical), batch, pos]
            kT2 = pers.tile([128, 2, S], BF)
            # v_aug[kpos, batch, block, dim+ones]
            v_aug = pers.tile([128, 2, NB, 65], BF)
            idn = pers.tile([128, 128], BF)
            make_identity(nc, idn)
            maskb = pers.tile([128, 128], BF)
            nc.sync.dma_start(out=maskb[:], in_=mb[:])
            nc.vector.memset(v_aug[:, :, :, 64:65], 1.0)
            # block-diagonal ones for the PE denominator broadcast:
            # rb = ones2.T @ [rec_h0; rec_h1] -> rows 0:64 = rec_h0, 64:128 =
            # rec_h1
            ones2 = pers.tile([2, 128], BF)
            nc.sync.dma_start(out=ones2[:], in_=ones2d[:])
            # gathered attention rows for o_proj (one tile per A2A)
            attg0 = pers.tile([128, 8, 512], BF)
            attg1 = pers.tile([128, 8, 512], BF)

            a2aA_in = dram.tile([NCORES, 128, 512], BF)
            a2aA_out = dram.tile([NCORES, 128, 512], BF)
            a2aB_in = dram.tile([NCORES, 128, 512], BF)
            a2aB_out = dram.tile([NCORES, 128, 512], BF)

            # ---------------- qkv projection + RoPE (bf16) ----------------
            with tc.tile_pool(name="proj", bufs=1) as pj, \
                 tc.tile_pool(name="projp", bufs=1, space="PSUM") as pjp:
                hsT_sb = pj.tile([128, 2, 17, S], BF)
                cos_sb = pj.tile([128, S], BF)
                sin_sb = pj.tile([128, S], BF)
                vT2 = pj.tile([128, S], BF)  # rows: [v(b1) | v(b0)]

                # startup loads spread over the 3 DMA trigger queues; order
                # matches consumption (kv(b0) first). wkv0 fully pre-fetched
                # first on gpsimd (bufs=17: no buffer reuse -> no pre-emission
                # race) so the first matmuls aren't queued behind hsT chunks.
                wkv0_sb = []
                for ki in range(17):
                    wt = pj.tile([128, 128], BF, tag="wkv0", bufs=17,
                                 name=f"wkv0_{ki}")
                    nc.gpsimd.dma_start(out=wt[:], in_=wkv0[ki])
                    wkv0_sb.append(wt)
                for ki in range(17):
                    eng = nc.sync if ki % 2 == 0 else nc.gpsimd
                    eng.dma_start(out=hsT_sb[:, 0, ki, :], in_=hsT[0, ki])
                for ki in range(17):
                    nc.scalar.dma_start(out=hsT_sb[:, 1, ki, :],
                                        in_=hsT[1, ki])
                nc.sync.dma_start(out=cos_sb[:], in_=cos2[:])
                nc.sync.dma_start(out=sin_sb[:], in_=sin2[:])

                def rope(pq, lo, hi, dst_slices, sl):
                    """RoPE rows [lo:hi) of psum pq; write to dst slices."""
                    n = hi - lo
                    tmp = pj.tile([128, 512], BF, tag="ropetmp", bufs=2)
                    pairs = [(0, 32), (32, 0), (64, 96), (96, 64)]
                    for (a, bb) in pairs:
                        if a >= n:
                            continue
                        nc.vector.tensor_tensor(
                            out=tmp[lo + a:lo + a + 32, :],
                            in0=pq[lo + bb:lo + bb + 32, :],
                            in1=sin_sb[lo + a:lo + a + 32, sl], op=MULT)
                    tmp2 = pj.tile([128, 512], BF, tag="ropetmp2", bufs=2)
                    nc.vector.tensor_tensor(out=tmp2[lo:hi, :],
                                            in0=pq[lo:hi, :],
                                            in1=cos_sb[lo:hi, sl], op=MULT)
                    for dst in dst_slices:
                        nc.vector.tensor_tensor(out=dst, in0=tmp2[lo:hi, :],
                                                in1=tmp[lo:hi, :], op=ADD)

                # kv units: b0 uses [k|v] weights, b1 uses [v|k]
                for b in range(2):
                    accs = [pjp.tile([128, 512], F32, tag="acc", bufs=6,
                                     name=f"acckv_{b}_{Q}") for Q in range(4)]
                    for ki in range(17):
                        if b == 0:
                            wt = wkv0_sb[ki]
                        else:
                            wt = pj.tile([128, 128], BF, tag="wkv1", bufs=4,
                                         name=f"wkv1_{ki}")
                            nc.gpsimd.dma_start(out=wt[:], in_=wkv1[ki])
                        for Q in range(4):
                            nc.tensor.matmul(
                                accs[Q][:], lhsT=wt[:],
                                rhs=hsT_sb[:, b, ki, 512 * Q:512 * Q + 512],
                                start=(ki == 0), stop=(ki == 16))
                    for Q in range(4):
                        sl = slice(512 * Q, 512 * Q + 512)
                        pq = accs[Q]
                        if b == 0:
                            # rows 0:64 = k(b0) -> kT2[0:64, 0]
                            rope(pq, 0, 64, [kT2[0:64, 0, sl]], sl)
                            nc.scalar.copy(vT2[64:128, sl], pq[64:128, :])
                        else:
                            # rows 64:128 = k(b1) -> kT2[64:128, 1]
                            rope(pq, 64, 128, [kT2[64:128, 1, sl]], sl)
                            nc.scalar.copy(vT2[0:64, sl], pq[0:64, :])
                    # duplicate kv head across partition halves
                    # (partition-shifted DVE copies, 512 cols each)
                    for Q in range(4):
                        sl = slice(512 * Q, 512 * Q + 512)
                        if b == 0:
                            nc.vector.tensor_copy(kT2[64:128, 0, sl],
                                                  kT2[0:64, 0, sl])
                        else:
                            nc.vector.tensor_copy(kT2[0:64, 1, sl],
                                                  kT2[64:128, 1, sl])

                # v: vT2 rows [v(b1)|v(b0)] -> v_aug[kpos, batch, block, dim]
                for kb in range(NB):
                    pvt = pjp.tile([128, 128], BF, tag="vt", bufs=2,
                                   name=f"pvt_{kb}")
                    nc.tensor.transpose(pvt[:], vT2[:, 128 * kb:128 * kb + 128],
                                        idn[:])
                    nc.vector.tensor_copy(v_aug[:, 1, kb, 0:64], pvt[:, 0:64])
                    nc.vector.tensor_copy(v_aug[:, 0, kb, 0:64],
                                          pvt[:, 64:128])

                # q units: pair p holds heads (2p, 2p+1) in halves.
                # Per-Q ki loops so rope(Q) overlaps the next Q's matmuls
                # instead of piling up at the unit end (hsT is fully
                # resident by the time q units run).
                wq_sb = {}
                for p in range(2):
                    for b in range(2):
                        for Q in range(4):
                            acc = pjp.tile([128, 512], F32, tag="acc",
                                           bufs=6, name=f"accq_{p}_{b}_{Q}")
                            for ki in range(17):
                                if (p, ki) not in wq_sb:
                                    wt = pj.tile([128, 128], BF, tag="wq",
                                                 bufs=17,
                                                 name=f"wq_{p}_{ki}")
                                    nc.scalar.dma_start(
                                        out=wt[:],
                                        in_=wq[ki, :, 128 * p:128 * p + 128])
                                    wq_sb[(p, ki)] = wt
                                nc.tensor.matmul(
                                    acc[:], lhsT=wq_sb[(p, ki)][:],
                                    rhs=hsT_sb[:, b, ki,
                                               512 * Q:512 * Q + 512],
                                    start=(ki == 0), stop=(ki == 16))
                            sl = slice(512 * Q, 512 * Q + 512)
                            rope(acc, 0, 128, [qT2[:, p, b, sl]], sl)

            # ---------------- attention ----------------
            with tc.tile_pool(name="att", bufs=1) as at, \
                 tc.tile_pool(name="attp", bufs=1, space="PSUM") as atp:

                def qk_strip(p, b, j, probsL, probsH):
                    q0 = 128 * j
                    L = LQ[j]
                    for cb in range(0, L, 1024):
                        w = min(1024, L - cb)
                        # alternate PE row groups (h0 at rows 0:64, h1 at
                        # 64:128) so consecutive matmuls co-execute
                        pss = [atp.tile([128, 1024], F32, tag="sc", bufs=3,
                                        name=f"sc_{p}_{b}_{j}_{cb}_{h}")
                               for h in (0, 1)]
                        for m0 in (0, 512):
                            if m0 >= w:
                                continue
                            mw = min(512, w - m0)
                            for h in (0, 1):
                                nc.tensor.matmul(
                                    pss[h][:, m0:m0 + mw],
                                    lhsT=kT2[64 * h:64 * h + 64, b,
                                             q0:q0 + 128],
                                    rhs=qT2[64 * h:64 * h + 64, p, b,
                                            q0 + cb + m0:q0 + cb + m0 + mw],
                                    start=True, stop=True)
                        for h, probs in ((0, probsL), (1, probsH)):
                            ps = pss[h]
                            dst = probs[:, OFFX[j] + cb:OFFX[j] + cb + w]
                            # engine split balances Act vs DVE load
                            if h == 0 or j in (8, 13, 14, 15):
                                nc.scalar.activation(dst, ps[:, 0:w], EXP,
                                                     scale=SCALE)
                            else:
                                nc.vector.tensor_scalar(
                                    out=dst.bitcast(I16), in0=ps[:, 0:w],
                                    scalar1=SCH_A, scalar2=SCH_B,
                                    op0=MULT, op1=ADD)
                            if cb == 0:
                                # causal mask on the diagonal block
                                nc.vector.tensor_tensor(
                                    out=probs[:, OFFX[j]:OFFX[j] + 128],
                                    in0=probs[:, OFFX[j]:OFFX[j] + 128],
                                    in1=maskb[:], op=MULT)

                def pv_chunk(p, b, h, c, probs, den):
                    """PV for 512-token chunk c; den row h of the pair's den
                    tile is filled; pvt copied to SBUF aa (releases PSUM)."""
                    pvt = atp.tile([65, 512], F32, tag="pv", bufs=2,
                                   name=f"pv_{p}_{b}_{h}_{c}")
                    for j in range(4 * c + 4):
                        if j <= 4 * c:
                            col = OFFX[j] + 512 * c - 128 * j
                            nc.tensor.matmul(
                                pvt[:, 0:512], lhsT=v_aug[:, b, j, :],
                                rhs=probs[:, col:col + 512],
                                start=(j == 0), stop=(j == 4 * c + 3))
                        else:
                            d0 = 128 * (j - 4 * c)
                            nc.tensor.matmul(
                                pvt[:, d0:512], lhsT=v_aug[:, b, j, :],
                                rhs=probs[:, OFFX[j]:OFFX[j] + 512 - d0],
                                start=False, stop=(j == 4 * c + 3))
                    aa = at.tile([64, 512], BF, tag="aa", bufs=4,
                                 name=f"aa_{p}_{b}_{h}_{c}")
                    nc.scalar.copy(aa[:], pvt[0:64, :])
                    nc.scalar.copy(den[0:1, h, :], pvt[64:65, :])
                    return aa

                def norm_send(p, b, c, den, aa0, aa1):
                    """Reciprocal + rank-1 PE broadcast of the denominators,
                    normalize, send to dest core 4b+c."""
                    rec = at.tile([1, 2, 512], F32, tag="rec", bufs=1,
                                  name=f"rec_{p}_{b}_{c}")
                    nc.vector.reciprocal_approx_fast(out=rec[:], in_=den[:])
                    rec_b = at.tile([1, 2, 512], BF, tag="recb", bufs=1,
                                    name=f"recb_{p}_{b}_{c}")
                    nc.scalar.copy(rec_b[:], rec[:])
                    a2a_in = a2aA_in if p == 0 else a2aB_in
                    for h, aa in ((0, aa0), (1, aa1)):
                        rb = atp.tile([64, 512], F32, tag="sc", bufs=3,
                                      name=f"rb_{p}_{b}_{h}_{c}")
                        nc.tensor.matmul(rb[:], lhsT=ones2[0:1, 0:64],
                                         rhs=rec_b[0:1, h, :],
                                         start=True, stop=True)
                        an = at.tile([64, 512], BF, tag="an", bufs=2,
                                     name=f"an_{p}_{b}_{h}_{c}")
                        nc.vector.tensor_tensor(
                            out=an[:], in0=aa[:], in1=rb[:], op=MULT)
                        nc.gpsimd.dma_start(
                            out=a2a_in[4 * b + c, 64 * h:64 * h + 64, :],
                            in_=an[:])

                pending = None
                for p in range(2):
                    for b in range(2):
                        probsL = at.tile([128, TOT], BF, tag="probsL", bufs=2,
                                         name=f"probsL_{p}_{b}")
                        probsH = at.tile([128, TOT], BF, tag="probsH", bufs=2,
                                         name=f"probsH_{p}_{b}")
                        # all QK strips first, then all PV chunks: PV never
                        # reads probs regions while exp is writing them, and
                        # norm_send is deferred one chunk so the rb matmul
                        # never stalls the in-order PE queue on recip
                        for j in range(NB):
                            qk_strip(p, b, j, probsL, probsH)
                            if j == 1 and pending is not None:
                                norm_send(*pending)
                                pending = None
                                if (p, b) == (1, 0):
                                    # pair 0 fully sent -> first AllToAll
                                    nc.gpsimd.collective_compute(
                                        "AllToAll", mybir.AluOpType.bypass,
                                        replica_groups=[list(range(NCORES))],
                                        ins=[a2aA_in.opt()],
                                        outs=[a2aA_out.opt()])
                                    nc.sync.dma_start(
                                        out=attg0[:],
                                        in_=a2aA_out.rearrange(
                                            "s p n -> p s n"))
                        for c in range(4):
                            den = at.tile([1, 2, 512], F32, tag="den",
                                          bufs=2, name=f"den_{p}_{b}_{c}")
                            aa0 = pv_chunk(p, b, 0, c, probsL, den)
                            aa1 = pv_chunk(p, b, 1, c, probsH, den)
                            if pending is not None:
                                norm_send(*pending)
                            pending = (p, b, c, den, aa0, aa1)
                norm_send(*pending)
                nc.gpsimd.collective_compute(
                    "AllToAll", mybir.AluOpType.bypass,
                    replica_groups=[list(range(NCORES))],
                    ins=[a2aB_in.opt()], outs=[a2aB_out.opt()])
                # readback on the (idle) sync queue so the A2A wait
                # doesn't block exp/copy work on other queues
                nc.sync.dma_start(
                    out=attg1[:],
                    in_=a2aB_out.rearrange("s p n -> p s n"))

            # ---------------- o_proj (my 512 tokens, all 2048 od) ---------
            # passes: (hf, od-half); wo streamed from DRAM; hf=0 GEMMs hide
            # A2A#2.
            with tc.tile_pool(name="op", bufs=1) as po, \
                 tc.tile_pool(name="opp", bufs=1, space="PSUM") as pop:
                part = {}
                for hf, attg in ((0, attg0), (1, attg1)):
                    for od in range(2):
                        ods = slice(1024 * od, 1024 * od + 1024)
                        psos = [pop.tile([128, 1024], F32, tag="po", bufs=4,
                                         name=f"pso_{hf}_{od}_{st}")
                                for st in range(4)]
                        for k8 in range(8):
                            wot = po.tile([128, 1024], BF, tag="wo", bufs=4,
                                          name=f"wo_{hf}_{od}_{k8}")
                            nc.scalar.dma_start(out=wot[:],
                                                in_=wo[8 * hf + k8, :, ods])
                            for st in range(4):
                                lhsT = attg[:, k8, 128 * st:128 * st + 128]
                                for u in range(2):
                                    nc.tensor.matmul(
                                        psos[st][:, 512 * u:512 * u + 512],
                                        lhsT=lhsT,
                                        rhs=wot[:, 512 * u:512 * u + 512],
                                        start=(k8 == 0), stop=(k8 == 7))
                        for st in range(4):
                            if hf == 0:
                                pt = po.tile([128, 1024], BF, tag="part",
                                             bufs=8, name=f"part_{od}_{st}")
                                nc.scalar.copy(pt[:], psos[st][:])
                                part[(od, st)] = pt
                            else:
                                oso = po.tile([128, 1024], BF, tag="oso",
                                              bufs=2, name=f"oso_{od}_{st}")
                                nc.vector.tensor_tensor(
                                    out=oso[:], in0=part[(od, st)][:],
                                    in1=psos[st][:], op=ADD)
                                nc.gpsimd.dma_start(
                                    out=out_part[128 * st:128 * st + 128,
                                                 ods],
                                    in_=oso[:])

    nc.compile()
    return nc


def _rope_tables():
    inv_freq = 1.0 / (ROPE_BASE ** (np.arange(0, HD, 2, dtype=np.float32) / HD))
    t = np.arange(S, dtype=np.float32)
    freqs = np.outer(t, inv_freq).astype(np.float32)  # [S, 32]
    cosT = np.cos(freqs).T  # [32, S]
    sinT = np.sin(freqs).T
    cos64 = np.concatenate([cosT, cosT], axis=0)          # [64, S]
    sin64 = np.concatenate([-sinT, sinT], axis=0)         # signed
    bf = ml_dtypes.bfloat16
    cos2 = np.concatenate([cos64, cos64], axis=0).astype(bf)
    sin2 = np.concatenate([sin64, sin64], axis=0).astype(bf)
    return cos2, sin2


def _np_reference(hidden_states, attention_mask, q_w, q_b, k_w, k_b, v_w, v_b,
                  o_w):
    hs = hidden_states.astype(np.float64)
    q = hs @ q_w.T.astype(np.float64) + q_b
    k = hs @ k_w.T.astype(np.float64) + k_b
    v = hs @ v_w.T.astype(np.float64) + v_b
    q = q.reshape(B, S, NH, HD).transpose(0, 2, 1, 3)
    k = k.reshape(B, S, NKV, HD).transpose(0, 2, 1, 3)
    v = v.reshape(B, S, NKV, HD).transpose(0, 2, 1, 3)
    inv_freq = 1.0 / (ROPE_BASE ** (np.arange(0, HD, 2) / HD))
    t = np.arange(S)
    freqs = np.outer(t, inv_freq)
    emb = np.concatenate([freqs, freqs], axis=-1)
    cos, sin = np.cos(emb), np.sin(emb)

    def rot(x):
        h = x.shape[-1] // 2
        return np.concatenate([-x[..., h:], x[..., :h]], axis=-1)

    q = q * cos + rot(q) * sin
    k = k * cos + rot(k) * sin
    k = np.repeat(k, GROUPS, axis=1)
    v = np.repeat(v, GROUPS, axis=1)
    sc = np.einsum("bhqd,bhkd->bhqk", q, k) / math.sqrt(HD)
    sc = sc + attention_mask.astype(np.float64)
    sc = sc - sc.max(axis=-1, keepdims=True)
    p = np.exp(sc)
    p = p / p.sum(axis=-1, keepdims=True)
    out = np.einsum("bhqk,bhkd->bhqd", p, v)
    out = out.transpose(0, 2, 1, 3).reshape(B, S, HID)
    return (out @ o_w.T.astype(np.float64)).astype(np.float32)


def _pack_chunks17(mat, bias):
    """[2048, M] weights + [M] bias -> [17, 128, M] with bias in row 0 of
    chunk 16."""
    m = mat.shape[1]
    out = np.zeros((17, 128, m), dtype=mat.dtype)
    out[:16] = mat.reshape(16, 128, m)
    out[16, 0, :] = bias
    return out


def _make_in_maps(inputs):
    hs = np.asarray(inputs["hidden_states"], np.float32)
    q_w = np.asarray(inputs["q_w"], np.float32)
    q_b = np.asarray(inputs["q_b"], np.float32)
    k_w = np.asarray(inputs["k_w"], np.float32)
    k_b = np.asarray(inputs["k_b"], np.float32)
    v_w = np.asarray(inputs["v_w"], np.float32)
    v_b = np.asarray(inputs["v_b"], np.float32)
    o_w = np.asarray(inputs["o_w"], np.float32)
    mask = np.asarray(inputs["attention_mask"], np.float32)
    m2 = mask[0, 0]

    bf = ml_dtypes.bfloat16
    cos2, sin2 = _rope_tables()
    # binary mask for the diagonal block, transposed orientation [k, q]
    maskbin = (m2[0:128, 0:128].T == 0.0).astype(bf)

    # wo rows in A2A arrival order: chunk 8*hf + s = heads (4s+2hf, 4s+2hf+1)
    o_wT = np.ascontiguousarray(o_w.T.astype(bf))  # [2048 in, 2048 out]
    rows = []
    for hf in range(2):
        for s in range(8):
            h = 4 * s + 2 * hf
            rows.extend(range(64 * h, 64 * h + 64))
            rows.extend(range(64 * (h + 1), 64 * (h + 1) + 64))
    wo_np = o_wT[np.array(rows)].reshape(16, 128, HID)

    hsT_packed = np.zeros((2, 17, 128, S), dtype=bf)
    for b in range(B):
        hsT_packed[b, :16] = np.ascontiguousarray(hs[b].T).astype(bf).reshape(
            16, 128, S)
        hsT_packed[b, 16, 0, :] = 1.0

    q_wT = np.ascontiguousarray(q_w.T).astype(bf)  # [2048, 2048]
    k_wT = np.ascontiguousarray(k_w.T).astype(bf)  # [2048, 512]
    v_wT = np.ascontiguousarray(v_w.T).astype(bf)

    # block-diagonal broadcast helper: rb = ones2.T @ [rec0; rec1]
    ones2_np = np.zeros((2, 128), dtype=bf)
    ones2_np[0, 0:64] = 1.0
    ones2_np[1, 64:128] = 1.0

    in_maps = []
    for c in range(NCORES):
        kv = slice(64 * c, 64 * c + 64)
        kvw0 = np.concatenate([k_wT[:, kv], v_wT[:, kv]], axis=1)
        kvw1 = np.concatenate([v_wT[:, kv], k_wT[:, kv]], axis=1)
        kvb0 = np.concatenate([k_b[kv], v_b[kv]]).astype(bf)
        kvb1 = np.concatenate([v_b[kv], k_b[kv]]).astype(bf)
        in_maps.append({
            "hsT": hsT_packed,
            "wq": _pack_chunks17(q_wT[:, 256 * c:256 * c + 256],
                                 q_b[256 * c:256 * c + 256].astype(bf)),
            "wkv0": _pack_chunks17(kvw0, kvb0),
            "wkv1": _pack_chunks17(kvw1, kvb1),
            "wo": wo_np,
            "cos2": cos2,
            "sin2": sin2,
            "maskbin": maskbin,
            "ones2": ones2_np,
        })
    return in_maps


def kernel(**inputs):
    mask = np.asarray(inputs["attention_mask"], np.float32)
    m2 = mask[0, 0]
    causal_ok = bool(
        np.all(m2[np.tril_indices(S)] == 0.0)
        and np.all(m2[np.triu_indices(S, 1)] < -1e8))
    if not causal_ok:
        return _np_reference(
            np.asarray(inputs["hidden_states"], np.float32), mask,
            *(np.asarray(inputs[k], np.float32)
              for k in ("q_w", "q_b", "k_w", "k_b", "v_w", "v_b", "o_w")))

    if "nc" not in _CACHED:
        _CACHED["nc"] = _build_nc()
    nc = _CACHED["nc"]
    in_maps = _make_in_maps(inputs)

    res = run_bass_kernel_spmd(nc, in_maps, list(range(NCORES)))

    out = np.empty((B, S, HID), dtype=np.float32)
    for c in range(NCORES):
        b, g = c // 4, c % 4
        out[b, 512 * g:512 * g + 512, :] = np.asarray(
            res.results[c]["out_part"], np.float32)
    return out
